# revision 22
# baseline (speedup 1.0000x reference)
"""Two-layer GAT (PyG-style GATConv x2) on 8 Trainium2 NeuronCores.

Design (v2, "host-expand"): nodes are sharded across the 8 cores by
destination. Between launches the HOST rearranges device-computed tables
(pure data movement: fancy-indexed row expansion per edge, sorting,
padding, hi/lo bf16 splits). All model arithmetic (matmuls, logit
add/leaky-relu/exp, softmax division, weighting, ELU, bias) runs on
device.

Rationale: per-edge SWDGE dma_gather costs ~8.3ns/edge of *serial* Q7
descriptor generation (~0.9ms/layer/core) - the measured bottleneck of
the v1 kernel. Pre-expanding edge payload rows on the host turns the
edge pass into dense sequential DMA + one-hot segment-sum matmuls.

Per-edge layout: edges are sorted by dst tile; each dst tile's edges are
padded to a multiple of 128 ("chunks"). Tiles are assigned to "slots" in
decreasing-count order per core so chunk counts align across the 8 SPMD
cores with minimal padding (the host un-permutes outputs).

Layer 1 packs 4 edges of the same dst node into one 260-wide row
(4 x (64 feats + w)), quartering the one-hot matmul count.

Three SPMD launches with host-side expansion between them:
  1. table0: h0^T = W0e^T @ x^T  -> feat-major table + per-node alphas
  2. layer-0 edges: stream payload/softmax/one-hot matmul -> ELU ->
     h1 = h0' @ W1e -> table1 (feat-major) + alphas
  3. layer-1 edges (quad-packed): same -> bias -> output shard
"""

import os

import numpy as np

import concourse.bacc as bacc
import concourse.mybir as mybir
from concourse import tile
from concourse.bass_utils import run_bass_kernel_spmd

fp32 = mybir.dt.float32
bf16 = mybir.dt.bfloat16
Alu = mybir.AluOpType
Act = mybir.ActivationFunctionType

NCORES = 8
NEG_SLOPE = 0.2
EPS = 1e-16
PAD_LOGIT = -30000.0
CPC = 16  # chunks per payload DMA call


def _dims():
    return dict(
        N=50000,
        NLOC=6250,
        NP=6272,  # padded to mult of 128
        NT=49,
        F_IN=256,
        HID=256,
        H=4,
        DH=64,
        C_OUT=64,
    )


# ---------------------------------------------------------------- launch 1


def build_l1(d):
    """h0^T = W0e^T @ x^T per core; W0e = [W0 | W0@A0] folds the per-node
    attention alphas into the same matmul. Outputs feat-major bf16 table
    plus fp32 alphas (host splits hi/lo)."""
    nc = bacc.Bacc(None, target_bir_lowering=False, debug=False)
    NP, F = d["NP"], d["F_IN"]

    xT = nc.dram_tensor("xT", [F, NP], bf16, kind="ExternalInput")
    W0e = nc.dram_tensor("W0e", [F, 264], bf16, kind="ExternalInput")
    t0T = nc.dram_tensor("t0T", [256, NP], bf16, kind="ExternalOutput")
    alT = nc.dram_tensor("alT", [8, NP], fp32, kind="ExternalOutput")

    TW = 512
    n_t = (NP + TW - 1) // TW

    with tile.TileContext(nc) as tc:
        with (
            tc.tile_pool(name="const", bufs=1) as cpool,
            tc.tile_pool(name="work", bufs=3) as pool,
            tc.tile_pool(name="psum", bufs=2, space="PSUM") as pp,
        ):
            w_sb = [
                cpool.tile([128, 264], bf16, tag=f"w{k}", name=f"w{k}")
                for k in range(2)
            ]
            for k in range(2):
                nc.sync.dma_start(w_sb[k][:], W0e[128 * k : 128 * (k + 1), :])

            for t in range(n_t):
                c0 = t * TW
                cw = min(TW, NP - c0)
                xt = [
                    pool.tile([128, TW], bf16, tag=f"xt{k}", name=f"xt{k}")
                    for k in range(2)
                ]
                for k in range(2):
                    nc.sync.dma_start(
                        xt[k][:, :cw], xT[128 * k : 128 * (k + 1), c0 : c0 + cw]
                    )
                for m in range(2):
                    ps = pp.tile([128, TW], fp32, tag=f"ps{m}", name=f"ps{m}")
                    for k in range(2):
                        nc.tensor.matmul(
                            ps[:, :cw],
                            w_sb[k][:, 128 * m : 128 * (m + 1)],
                            xt[k][:, :cw],
                            start=(k == 0),
                            stop=(k == 1),
                        )
                    ob = pool.tile([128, TW], bf16, tag=f"ob{m}", name=f"ob{m}")
                    nc.scalar.activation(ob[:, :cw], ps[:, :cw], Act.Copy)
                    nc.sync.dma_start(
                        t0T[128 * m : 128 * (m + 1), c0 : c0 + cw], ob[:, :cw]
                    )
                pa = pp.tile([8, TW], fp32, tag="pa", name="pa")
                for k in range(2):
                    nc.tensor.matmul(
                        pa[:, :cw],
                        w_sb[k][:, 256:264],
                        xt[k][:, :cw],
                        start=(k == 0),
                        stop=(k == 1),
                    )
                oa = pool.tile([8, TW], fp32, tag="oa", name="oa")
                nc.scalar.activation(oa[:, :cw], pa[:, :cw], Act.Copy)
                nc.sync.dma_start(alT[:, c0 : c0 + cw], oa[:, :cw])
    nc.compile()
    return nc


# ------------------------------------------------------------ edge machinery


def _logits_phase(nc, tc, d, L, NCH, ewb):
    """Batched per-edge softmax numerators: ewb = exp(lrelu(as+ad)) from
    hi/lo bf16 pieces, computed up-front for all chunks."""
    NBLK = 2
    nb = (NCH + NBLK - 1) // NBLK
    with tc.tile_pool(name="logit", bufs=2) as pool:
        for b in range(NBLK):
            b0 = b * nb
            bw = min(nb, NCH - b0)
            if bw <= 0:
                break
            lb = pool.tile([128, nb, 16], bf16, tag="lb", name="lb")
            nc.sync.dma_start(lb[:, :bw, :], L[:, b0 : b0 + bw, :])
            e8 = pool.tile([128, nb, 8], fp32, tag="e8", name="e8")
            nc.vector.tensor_tensor(
                e8[:, :bw, :], lb[:, :bw, 0:8], lb[:, :bw, 8:16], op=Alu.add
            )
            e4 = pool.tile([128, nb, 4], fp32, tag="e4", name="e4")
            nc.vector.tensor_tensor(
                e4[:, :bw, :], e8[:, :bw, 0:4], e8[:, :bw, 4:8], op=Alu.add
            )
            nc.vector.scalar_tensor_tensor(
                e4[:, :bw, :],
                e4[:, :bw, :],
                NEG_SLOPE,
                e4[:, :bw, :],
                op0=Alu.mult,
                op1=Alu.max,
            )
            nc.scalar.activation(ewb[:, b0 : b0 + bw, :], e4[:, :bw, :], Act.Exp)


def _edge_pass(nc, tc, d, P, Ks, ewb, rr_sb, iota_sb, fin, pp):
    """Stream pre-expanded 260-wide payload rows (4 blocks x (64 feats +
    w-slot)), weight by ewb, one-hot segment-sum into per-tile PSUM."""
    NCH = sum(Ks)

    with tc.tile_pool(name="edge", bufs=3) as pool:
        state = dict(ncalls=0, tiles={})

        def emit_call(call):
            c0 = call * CPC
            nch = min(CPC, NCH - c0)
            G = pool.tile([128, CPC, 264], bf16, tag="G", name="G", bufs=6)
            OH = pool.tile([128, CPC, 128], bf16, tag="OH", name="OH", bufs=6)
            eng = nc.sync if call % 2 == 0 else nc.scalar
            eng.dma_start(G[:, :nch, :], P[:, c0 : c0 + nch, :])
            rb = rr_sb[:, c0 : c0 + nch].unsqueeze(2).broadcast_to([128, nch, 128])
            ib = iota_sb[:].unsqueeze(1).broadcast_to([128, nch, 128])
            # iota first: in0 is the packed-readable operand, which lets the
            # DVE pick 2x_1P; the broadcast rr rides port 1
            nc.vector.tensor_tensor(OH[:, :nch, :], ib, rb, op=Alu.is_equal)
            g4 = G[:, :nch, :].rearrange("p c (h e) -> p c h e", e=66)
            wb = (
                ewb[:, c0 : c0 + nch, :]
                .unsqueeze(3)
                .broadcast_to([128, nch, 4, 66])
            )
            # payload w-slots are 1.0 from the host, so this multiply also
            # writes the per-block softmax-denominator columns
            nc.vector.tensor_tensor(g4, g4, wb, op=Alu.mult)
            return G, OH

        c = 0
        for s in range(len(Ks)):
            ps = pp.tile([128, 264], fp32, tag="ps", name="ps", bufs=4)
            for k in range(Ks[s]):
                call, cin = c // CPC, c % CPC
                if call >= state["ncalls"]:
                    state["tiles"][call] = emit_call(call)
                    state["ncalls"] = call + 1
                    state["tiles"].pop(call - 5, None)
                G, OH = state["tiles"][call]
                nc.tensor.matmul(
                    ps[:],
                    OH[:, cin, :],
                    G[:, cin, :],
                    start=(k == 0),
                    stop=(k == Ks[s] - 1),
                )
                c += 1
            fin(s, ps)


# ---------------------------------------------------------------- launch 2


def build_l2(d, Ks):
    """Layer-0 edge pass (softmax-div + bias + ELU fused in finalize),
    then table1^T = W1e^T @ h0'^T via a DMA-transpose round trip."""
    nc = bacc.Bacc(None, target_bir_lowering=False, debug=False)
    NP, NT, H = d["NP"], d["NT"], d["H"]
    NCH = sum(Ks)

    P = nc.dram_tensor("P", [128, NCH, 264], bf16, kind="ExternalInput")
    L = nc.dram_tensor("L", [128, NCH, 16], bf16, kind="ExternalInput")
    RR = nc.dram_tensor("RR", [128, NCH], bf16, kind="ExternalInput")
    IOTA = nc.dram_tensor("IOTA", [128, 128], bf16, kind="ExternalInput")
    W1e = nc.dram_tensor("W1e", [256, 66], bf16, kind="ExternalInput")
    B0 = nc.dram_tensor("B0", [128, 256], bf16, kind="ExternalInput")
    t1T = nc.dram_tensor("t1T", [64, NP], bf16, kind="ExternalOutput")
    a1T = nc.dram_tensor("a1T", [2, NP], fp32, kind="ExternalOutput")

    with tile.TileContext(nc) as tc:
        with (
            tc.tile_pool(name="const", bufs=1) as cpool,
            tc.tile_pool(name="persist", bufs=1) as ipool,
            tc.tile_pool(name="fin", bufs=3) as fpool,
            tc.tile_pool(name="psum", bufs=1, space="PSUM") as pp,
        ):
            iota_sb = cpool.tile([128, 128], bf16)
            nc.sync.dma_start(iota_sb[:], IOTA[:])
            b0_sb = cpool.tile([128, 256], bf16)
            nc.sync.dma_start(b0_sb[:], B0[:])
            rr_sb = ipool.tile([128, NCH], bf16)
            nc.sync.dma_start(rr_sb[:], RR[:])
            ewb = ipool.tile([128, NCH, 4], bf16)
            H0 = ipool.tile([128, NT, 256], bf16)

            _logits_phase(nc, tc, d, L, NCH, ewb)

            def fin0(s, ps):
                sb = fpool.tile([128, 264], fp32, tag="sb", name="sb")
                nc.scalar.activation(sb[:], ps[:], Act.Copy)
                pv = sb[:].rearrange("p (h e) -> p h e", h=H)
                dn = fpool.tile([128, H], fp32, tag="dn", name="dn")
                nc.vector.tensor_scalar_add(dn[:], pv[:, :, 64], EPS)
                rec = fpool.tile([128, H], fp32, tag="rec", name="rec")
                nc.vector.reciprocal(rec[:], dn[:])
                xp = fpool.tile([128, 256], bf16, tag="xp", name="xp")
                rb = rec[:].unsqueeze(2).broadcast_to([128, H, 64])
                nc.vector.tensor_tensor(
                    xp[:].rearrange("p (h e) -> p h e", h=H),
                    pv[:, :, 0:64],
                    rb,
                    op=Alu.mult,
                )
                z = fpool.tile([128, 256], bf16, tag="z", name="z")
                nc.vector.tensor_tensor(z[:], xp[:], b0_sb[:], op=Alu.add)
                ex = fpool.tile([128, 256], fp32, tag="ex", name="ex")
                nc.scalar.activation(ex[:], z[:], Act.Exp, bias=1.0)
                m1 = fpool.tile([128, 256], fp32, tag="m1", name="m1")
                nc.vector.tensor_scalar_min(m1[:], ex[:], 1.0)
                nc.vector.scalar_tensor_tensor(
                    H0[:, s, :], z[:], -1.0, m1[:], op0=Alu.max, op1=Alu.add
                )

            _edge_pass(nc, tc, d, P, Ks, ewb, rr_sb, iota_sb, fin0, pp)

            with (
                tc.tile_pool(name="tb1", bufs=3) as tpool,
                tc.tile_pool(name="dram", bufs=1, space="DRAM") as dpool,
                tc.tile_pool(name="tb1psum", bufs=3, space="PSUM") as pp1,
            ):
                h0d = dpool.tile([NP, 256], bf16)
                nc.sync.dma_start(
                    h0d[:].rearrange("(t p) f -> p t f", p=128), H0[:, :, :]
                )
                h0T = [
                    ipool.tile([128, NP], bf16, tag=f"h0T{k}", name=f"h0T{k}")
                    for k in range(2)
                ]
                for k in range(2):
                    nc.sync.dma_start_transpose(
                        h0T[k][:], h0d[:, 128 * k : 128 * (k + 1)]
                    )
                w1_sb = [
                    cpool.tile([128, 66], bf16, tag=f"w1_{k}", name=f"w1_{k}")
                    for k in range(2)
                ]
                for k in range(2):
                    nc.sync.dma_start(w1_sb[k][:], W1e[128 * k : 128 * (k + 1), :])
                TW = 512
                for j in range((NP + TW - 1) // TW):
                    c0 = j * TW
                    cw = min(TW, NP - c0)
                    pt = pp1.tile([66, TW], fp32, tag="pt", name="pt")
                    for k in range(2):
                        nc.tensor.matmul(
                            pt[:, :cw],
                            w1_sb[k][:],
                            h0T[k][:, c0 : c0 + cw],
                            start=(k == 0),
                            stop=(k == 1),
                        )
                    tb = tpool.tile([64, TW], bf16, tag="tb", name="tb")
                    nc.scalar.activation(tb[:, :cw], pt[0:64, :cw], Act.Copy)
                    nc.sync.dma_start(t1T[:, c0 : c0 + cw], tb[:, :cw])
                    ab = tpool.tile([2, TW], fp32, tag="ab", name="ab")
                    nc.scalar.activation(ab[:, :cw], pt[64:66, :cw], Act.Copy)
                    nc.sync.dma_start(a1T[:, c0 : c0 + cw], ab[:, :cw])
    nc.compile()
    return nc


# ---------------------------------------------------------------- launch 3


def build_l3(d, Ks):
    """Layer-1 edge pass, quad-packed (4 same-dst edges per 260-wide row);
    finalize = sum quads, softmax-div, bias."""
    nc = bacc.Bacc(None, target_bir_lowering=False, debug=False)
    NP, C = d["NP"], d["C_OUT"]
    NCH = sum(Ks)

    P = nc.dram_tensor("P", [128, NCH, 264], bf16, kind="ExternalInput")
    L = nc.dram_tensor("L", [128, NCH, 16], bf16, kind="ExternalInput")
    RR = nc.dram_tensor("RR", [128, NCH], bf16, kind="ExternalInput")
    IOTA = nc.dram_tensor("IOTA", [128, 128], bf16, kind="ExternalInput")
    B1 = nc.dram_tensor("B1", [128, C], fp32, kind="ExternalInput")
    out = nc.dram_tensor("out", [NP, C], fp32, kind="ExternalOutput")

    with tile.TileContext(nc) as tc:
        with (
            tc.tile_pool(name="const", bufs=1) as cpool,
            tc.tile_pool(name="persist", bufs=1) as ipool,
            tc.tile_pool(name="fin", bufs=3) as fpool,
            tc.tile_pool(name="psum", bufs=1, space="PSUM") as pp,
        ):
            iota_sb = cpool.tile([128, 128], bf16)
            nc.sync.dma_start(iota_sb[:], IOTA[:])
            b1_sb = cpool.tile([128, C], fp32)
            nc.sync.dma_start(b1_sb[:], B1[:])
            rr_sb = ipool.tile([128, NCH], bf16)
            nc.sync.dma_start(rr_sb[:], RR[:])
            ewb = ipool.tile([128, NCH, 4], bf16)

            _logits_phase(nc, tc, d, L, NCH, ewb)

            def fin1(s, ps):
                sb = fpool.tile([128, 264], fp32, tag="sb", name="sb")
                nc.scalar.activation(sb[:], ps[:], Act.Copy)
                sv = sb[:].rearrange("p (q e) -> p q e", q=4)
                a01 = fpool.tile([128, 66], fp32, tag="a01", name="a01")
                nc.vector.tensor_tensor(a01[:], sv[:, 0, :], sv[:, 1, :], op=Alu.add)
                a23 = fpool.tile([128, 66], fp32, tag="a23", name="a23")
                nc.vector.tensor_tensor(a23[:], sv[:, 2, :], sv[:, 3, :], op=Alu.add)
                tot = fpool.tile([128, 66], fp32, tag="tot", name="tot")
                nc.vector.tensor_tensor(tot[:], a01[:], a23[:], op=Alu.add)
                dn = fpool.tile([128, 1], fp32, tag="dnq", name="dnq")
                nc.vector.tensor_scalar_add(dn[:], tot[:, 64:65], EPS)
                rec = fpool.tile([128, 1], fp32, tag="recq", name="recq")
                nc.vector.reciprocal(rec[:], dn[:])
                O = fpool.tile([128, C], fp32, tag="O", name="O")
                nc.vector.scalar_tensor_tensor(
                    O[:], tot[:, 0:64], rec[:], b1_sb[:], op0=Alu.mult, op1=Alu.add
                )
                nc.sync.dma_start(out[128 * s : 128 * (s + 1), :], O[:])

            _edge_pass(nc, tc, d, P, Ks, ewb, rr_sb, iota_sb, fin1, pp)
    nc.compile()
    return nc


# ------------------------------------------------------------ host plumbing


def _bf16(a):
    import ml_dtypes

    return np.asarray(a).astype(ml_dtypes.bfloat16)


def _hilo(a):
    """fp32 array -> (hi, lo) bf16 with hi+lo ~= a."""
    hi = _bf16(a)
    lo = _bf16(a - hi.astype(np.float32))
    return hi, lo


def _build_A0(att_src, att_dst):
    H, DH = att_src.shape
    A = np.zeros((H * DH, 2 * H), np.float32)
    for h in range(H):
        A[h * DH : (h + 1) * DH, h] = att_src[h]
        A[h * DH : (h + 1) * DH, H + h] = att_dst[h]
    return A


def _prep_edges(edge_index, d):
    """Per-core slot structure for both layers.

    l2 (per-edge): slots = dst tiles sorted by edge count (desc) per core;
    K2[s] = max over cores of ceil(count/128).
    l3 (quad): 4 same-dst edges per row; slots = tiles sorted by quad
    count. Returns per-core index arrays into the node tables.
    """
    N, NLOC, NT = d["N"], d["NLOC"], d["NT"]
    src = np.concatenate([edge_index[0], np.arange(N, dtype=np.int64)])
    dst = np.concatenate([edge_index[1], np.arange(N, dtype=np.int64)])
    core = dst // NLOC

    percore = []
    for c in range(NCORES):
        m = core == c
        s_c, t_c = src[m], dst[m] - c * NLOC
        order = np.argsort(t_c, kind="stable")
        percore.append((s_c[order], t_c[order]))

    # ---- layer-0 structure (per edge)
    counts2 = np.zeros((NCORES, NT), np.int64)
    for c in range(NCORES):
        counts2[c] = np.bincount(percore[c][1] // 128, minlength=NT)
    perm2 = np.argsort(-counts2, axis=1, kind="stable")  # [core, slot] -> tile
    sorted2 = -np.sort(-counts2, axis=1)
    K2 = tuple(int(k) for k in np.ceil(sorted2.max(axis=0) / 128).astype(int))
    NCH2 = sum(K2)
    base2 = np.concatenate([[0], np.cumsum(np.array(K2) * 128)])

    l2 = []
    for c in range(NCORES):
        s_c, t_c = percore[c]
        tile_of = t_c // 128
        EP = NCH2 * 128
        gsrc = np.zeros(EP, np.int64)
        gdst = np.zeros(EP, np.int64)
        rr = np.full(EP, -1.0, np.float32)
        pad = np.ones(EP, bool)
        offs = np.concatenate([[0], np.cumsum(counts2[c][perm2[c]])])
        # edges are tile-sorted; index ranges per tile:
        tstart = np.concatenate([[0], np.cumsum(counts2[c])])
        for s in range(NT):
            tl = perm2[c][s]
            n = counts2[c][tl]
            sl = slice(tstart[tl], tstart[tl] + n)
            b = base2[s]
            gsrc[b : b + n] = s_c[sl]
            gdst[b : b + n] = t_c[sl] + c * NLOC
            rr[b : b + n] = (t_c[sl] - 128 * tl).astype(np.float32)
            pad[b : b + n] = False
        l2.append(dict(gsrc=gsrc, gdst=gdst, rr=rr, pad=pad))

    # ---- layer-1 structure (quads)
    counts3 = np.zeros((NCORES, NT), np.int64)
    quads_pc = []
    for c in range(NCORES):
        s_c, t_c = percore[c]
        deg = np.bincount(t_c, minlength=NLOC)
        nq = (deg + 3) // 4  # quads per node
        counts3[c] = np.add.reduceat(
            nq, np.arange(0, NLOC, 128)
        )
        quads_pc.append((s_c, t_c, deg, nq))
    perm3 = np.argsort(-counts3, axis=1, kind="stable")
    sorted3 = -np.sort(-counts3, axis=1)
    K3 = tuple(int(k) for k in np.ceil(sorted3.max(axis=0) / 128).astype(int))
    NCH3 = sum(K3)
    base3 = np.concatenate([[0], np.cumsum(np.array(K3) * 128)])

    l3 = []
    for c in range(NCORES):
        s_c, t_c, deg, nq = quads_pc[c]
        EP = NCH3 * 128
        qsrc = np.zeros((EP, 4), np.int64)
        qdst = np.zeros(EP, np.int64)
        rr = np.full(EP, -1.0, np.float32)
        pad = np.ones((EP, 4), bool)
        estart = np.concatenate([[0], np.cumsum(deg)])
        qstart_tile = np.concatenate(
            [[0], np.cumsum(counts3[c])]
        )  # quad offset per tile (in tile order)
        for s in range(NT):
            tl = perm3[c][s]
            b = base3[s]
            q = 0
            n0 = tl * 128
            n1 = min(n0 + 128, NLOC)
            for node in range(n0, n1):
                dg = deg[node]
                if dg == 0:
                    continue
                e0 = estart[node]
                nqn = nq[node]
                rows = b + q + np.arange(nqn)
                rr[rows] = float(node - n0)
                qdst[rows] = node + c * NLOC
                es = s_c[e0 : e0 + dg]
                full = np.zeros(nqn * 4, np.int64)
                full[:dg] = es
                qsrc[rows] = full.reshape(nqn, 4)
                pd = np.ones(nqn * 4, bool)
                pd[:dg] = False
                pad[rows] = pd.reshape(nqn, 4)
                q += nqn
        l3.append(dict(qsrc=qsrc, qdst=qdst, rr=rr, pad=pad))

    return dict(K2=K2, K3=K3, perm2=perm2, perm3=perm3, l2=l2, l3=l3)


def _pack_pm(a, nch):
    """[EP, W] row-major -> [128, nch, W] partition-major contiguous."""
    W = a.shape[1]
    return np.ascontiguousarray(a.reshape(nch, 128, W).transpose(1, 0, 2))


def _expand_l2(core_idx, tab0, a0, prep):
    """Per-core launch-2 inputs from full node tables (pure gather)."""
    K2 = prep["K2"]
    NCH = sum(K2)
    e = prep["l2"][core_idx]
    gsrc, gdst, pad = e["gsrc"], e["gdst"], e["pad"]
    EP = NCH * 128
    rows = tab0[gsrc]  # [EP, 256] bf16
    P = np.zeros((EP, 264), rows.dtype)
    pv = P.reshape(EP, 4, 66)
    pv[:, :, 0:64] = rows.reshape(EP, 4, 64)
    pv[:, :, 64] = 1.0  # weighting writes w into these denominator slots
    as_hi, as_lo = a0["as_hi"][gsrc], a0["as_lo"][gsrc]
    ad_hi, ad_lo = a0["ad_hi"][gdst], a0["ad_lo"][gdst]
    L = np.concatenate([as_hi, as_lo, ad_hi, ad_lo], axis=1)
    L[pad, 0:4] = PAD_LOGIT
    rr = e["rr"].reshape(NCH, 128).T
    return dict(
        P=_pack_pm(P, NCH),
        L=_pack_pm(L, NCH),
        RR=np.ascontiguousarray(_bf16(rr)),
    )


def _expand_l3(core_idx, tab1, a1, prep):
    K3 = prep["K3"]
    NCH = sum(K3)
    e = prep["l3"][core_idx]
    qsrc, qdst, pad = e["qsrc"], e["qdst"], e["pad"]
    EP = NCH * 128
    P = np.zeros((EP, 264), tab1.dtype)
    pv = P.reshape(EP, 4, 66)
    for j in range(4):
        pv[:, j, 0:64] = tab1[qsrc[:, j]]
    pv[:, :, 64] = 1.0  # weighting writes w into these denominator slots
    as_hi = a1["as_hi"][qsrc]  # [EP, 4]
    as_lo = a1["as_lo"][qsrc]
    ad_hi = np.repeat(a1["ad_hi"][qdst][:, None], 4, axis=1)
    ad_lo = np.repeat(a1["ad_lo"][qdst][:, None], 4, axis=1)
    L = np.concatenate([as_hi, as_lo, ad_hi, ad_lo], axis=1)
    L[:, 0:4][pad] = PAD_LOGIT
    rr = e["rr"].reshape(NCH, 128).T
    return dict(
        P=_pack_pm(P, NCH),
        L=_pack_pm(L, NCH),
        RR=np.ascontiguousarray(_bf16(rr)),
    )


_cache = {}
LAST_PROFILE = {}


def _run(nc, in_maps, core_ids, label):
    trace = bool(int(os.environ.get("GAT_PROFILE", "0")))
    if trace:
        try:
            import sys

            import profile_hook

            profile_hook.install()
            import concourse.bass_utils as bu

            bu.upload_artifacts = lambda tmpdir: "local://skipped"
            tdir = f"/tmp/gat_trace_{label}"
            os.makedirs(tdir, exist_ok=True)
            for f in os.listdir(tdir):
                os.unlink(os.path.join(tdir, f))
            br = run_bass_kernel_spmd(nc, in_maps, core_ids, trace=True, tmpdir=tdir)
            LAST_PROFILE[label] = br.exec_time_ns
            return br.results
        except Exception as e:  # fall back to untraced
            print(f"traced run failed ({e!r}); untraced retry", file=sys.stderr)
    br = run_bass_kernel_spmd(nc, in_maps, core_ids)
    LAST_PROFILE[label] = br.exec_time_ns
    return br.results


def kernel(x, edge_index, W0, att_src0, att_dst0, b0, W1, att_src1, att_dst1, b1):
    x = np.asarray(x, np.float32)
    edge_index = np.asarray(edge_index)
    d = _dims()
    N, NLOC, NP, NT = d["N"], d["NLOC"], d["NP"], d["NT"]

    prep = _prep_edges(edge_index, d)
    key = (prep["K2"], prep["K3"])
    if key not in _cache:
        _cache[key] = (build_l1(d), build_l2(d, prep["K2"]), build_l3(d, prep["K3"]))
    nc1, nc2, nc3 = _cache[key]

    A0 = _build_A0(np.asarray(att_src0), np.asarray(att_dst0))
    W0f = np.asarray(W0, np.float32)
    W0e = _bf16(np.concatenate([W0f, W0f @ A0], axis=1))
    W1f = np.asarray(W1, np.float32)
    was1 = W1f @ np.asarray(att_src1, np.float32).ravel()
    wad1 = W1f @ np.asarray(att_dst1, np.float32).ravel()
    W1e = _bf16(np.stack([*W1f.T, was1, wad1], axis=1))  # [256, 66]
    b0m1 = np.tile(np.asarray(b0, np.float32)[None, :] - 1.0, (128, 1))
    b1r = np.tile(np.asarray(b1, np.float32)[None, :], (128, 1))
    iota = _bf16(np.tile(np.arange(128, dtype=np.float32)[None, :], (128, 1)))
    core_ids = list(range(NCORES))

    # launch 1
    xb = _bf16(x)
    in1 = []
    for c in range(NCORES):
        xT = np.zeros((d["F_IN"], NP), xb.dtype)
        xT[:, :NLOC] = xb[c * NLOC : (c + 1) * NLOC].T
        in1.append(dict(xT=xT, W0e=W0e))
    r1 = _run(nc1, in1, core_ids, "l1")

    tab0 = np.ascontiguousarray(
        np.concatenate(
            [r1[c]["t0T"][:, :NLOC] for c in range(NCORES)], axis=1
        ).T
    )  # [N, 256] bf16
    alf = np.concatenate([r1[c]["alT"][:, :NLOC] for c in range(NCORES)], axis=1)
    as_hi, as_lo = _hilo(alf[0:4].T)
    ad_hi, ad_lo = _hilo(alf[4:8].T)
    a0 = dict(as_hi=as_hi, as_lo=as_lo, ad_hi=ad_hi, ad_lo=ad_lo)

    in2 = [
        dict(
            _expand_l2(c, tab0, a0, prep),
            IOTA=iota,
            W1e=W1e,
            B0=_bf16(b0m1),
        )
        for c in range(NCORES)
    ]
    r2 = _run(nc2, in2, core_ids, "l2")

    # un-permute slot-major table1 columns -> node order
    tab1 = np.zeros((N, 64), r2[0]["t1T"].dtype)
    a1sh = np.zeros(N, np.float32)
    a1dh = np.zeros(N, np.float32)
    for c in range(NCORES):
        t1 = r2[c]["t1T"]  # [64, NP] slot-major
        a1c = r2[c]["a1T"]  # [2, NP]
        for s in range(NT):
            tl = prep["perm2"][c][s]
            n0 = tl * 128
            n1 = min(n0 + 128, NLOC)
            w = n1 - n0
            if w <= 0:
                continue
            tab1[c * NLOC + n0 : c * NLOC + n1] = t1[:, 128 * s : 128 * s + w].T
            a1sh[c * NLOC + n0 : c * NLOC + n1] = a1c[0, 128 * s : 128 * s + w]
            a1dh[c * NLOC + n0 : c * NLOC + n1] = a1c[1, 128 * s : 128 * s + w]
    s_hi, s_lo = _hilo(a1sh)
    d_hi, d_lo = _hilo(a1dh)
    a1 = dict(as_hi=s_hi, as_lo=s_lo, ad_hi=d_hi, ad_lo=d_lo)

    in3 = [
        dict(_expand_l3(c, tab1, a1, prep), IOTA=iota, B1=b1r)
        for c in range(NCORES)
    ]
    r3 = _run(nc3, in3, core_ids, "l3")

    out = np.zeros((N, 64), np.float32)
    for c in range(NCORES):
        o = r3[c]["out"]  # [NP, 64] slot-major
        for s in range(NT):
            tl = prep["perm3"][c][s]
            n0 = tl * 128
            n1 = min(n0 + 128, NLOC)
            w = n1 - n0
            if w <= 0:
                continue
            out[c * NLOC + n0 : c * NLOC + n1] = o[128 * s : 128 * s + w]
    return out


# revision 23
# speedup vs baseline: 1.1752x; 1.1752x over previous
"""Two-layer GAT (PyG-style GATConv x2) on 8 Trainium2 NeuronCores.

Design (v2, "host-expand"): nodes are sharded across the 8 cores by
destination. Between launches the HOST rearranges device-computed tables
(pure data movement: fancy-indexed row expansion per edge, sorting,
padding, hi/lo bf16 splits). All model arithmetic (matmuls, logit
add/leaky-relu/exp, softmax division, weighting, ELU, bias) runs on
device.

Rationale: per-edge SWDGE dma_gather costs ~8.3ns/edge of *serial* Q7
descriptor generation (~0.9ms/layer/core) - the measured bottleneck of
the v1 kernel. Pre-expanding edge payload rows on the host turns the
edge pass into dense sequential DMA + one-hot segment-sum matmuls.

Per-edge layout: edges are sorted by dst tile; each dst tile's edges are
padded to a multiple of 128 ("chunks"). Tiles are assigned to "slots" in
decreasing-count order per core so chunk counts align across the 8 SPMD
cores with minimal padding (the host un-permutes outputs).

Layer 1 packs 4 edges of the same dst node into one 260-wide row
(4 x (64 feats + w)), quartering the one-hot matmul count.

Three SPMD launches with host-side expansion between them:
  1. table0: h0^T = W0e^T @ x^T  -> feat-major table + per-node alphas
  2. layer-0 edges: stream payload/softmax/one-hot matmul -> ELU ->
     h1 = h0' @ W1e -> table1 (feat-major) + alphas
  3. layer-1 edges (quad-packed): same -> bias -> output shard
"""

import os

import numpy as np

import concourse.bacc as bacc
import concourse.mybir as mybir
from concourse import tile
from concourse.bass_utils import run_bass_kernel_spmd

fp32 = mybir.dt.float32
bf16 = mybir.dt.bfloat16
Alu = mybir.AluOpType
Act = mybir.ActivationFunctionType

NCORES = 8
NEG_SLOPE = 0.2
EPS = 1e-16
PAD_LOGIT = -30000.0
CPC = 16  # chunks per payload DMA call


def _dims():
    return dict(
        N=50000,
        NLOC=6250,
        NP=6272,  # padded to mult of 128
        NT=49,
        F_IN=256,
        HID=256,
        H=4,
        DH=64,
        C_OUT=64,
    )


# ---------------------------------------------------------------- launch 1


def build_l1(d):
    """h0^T = W0e^T @ x^T per core; W0e = [W0 | W0@A0] folds the per-node
    attention alphas into the same matmul. Outputs feat-major bf16 table
    plus fp32 alphas (host splits hi/lo)."""
    nc = bacc.Bacc(None, target_bir_lowering=False, debug=False)
    NP, F = d["NP"], d["F_IN"]

    xT = nc.dram_tensor("xT", [F, NP], bf16, kind="ExternalInput")
    W0e = nc.dram_tensor("W0e", [F, 264], bf16, kind="ExternalInput")
    t0T = nc.dram_tensor("t0T", [256, NP], bf16, kind="ExternalOutput")
    alT = nc.dram_tensor("alT", [8, NP], fp32, kind="ExternalOutput")

    TW = 512
    n_t = (NP + TW - 1) // TW

    with tile.TileContext(nc) as tc:
        with (
            tc.tile_pool(name="const", bufs=1) as cpool,
            tc.tile_pool(name="work", bufs=3) as pool,
            tc.tile_pool(name="psum", bufs=2, space="PSUM") as pp,
        ):
            w_sb = [
                cpool.tile([128, 264], bf16, tag=f"w{k}", name=f"w{k}")
                for k in range(2)
            ]
            for k in range(2):
                nc.sync.dma_start(w_sb[k][:], W0e[128 * k : 128 * (k + 1), :])

            for t in range(n_t):
                c0 = t * TW
                cw = min(TW, NP - c0)
                xt = [
                    pool.tile([128, TW], bf16, tag=f"xt{k}", name=f"xt{k}")
                    for k in range(2)
                ]
                for k in range(2):
                    nc.sync.dma_start(
                        xt[k][:, :cw], xT[128 * k : 128 * (k + 1), c0 : c0 + cw]
                    )
                for m in range(2):
                    ps = pp.tile([128, TW], fp32, tag=f"ps{m}", name=f"ps{m}")
                    for k in range(2):
                        nc.tensor.matmul(
                            ps[:, :cw],
                            w_sb[k][:, 128 * m : 128 * (m + 1)],
                            xt[k][:, :cw],
                            start=(k == 0),
                            stop=(k == 1),
                        )
                    ob = pool.tile([128, TW], bf16, tag=f"ob{m}", name=f"ob{m}")
                    nc.scalar.activation(ob[:, :cw], ps[:, :cw], Act.Copy)
                    nc.sync.dma_start(
                        t0T[128 * m : 128 * (m + 1), c0 : c0 + cw], ob[:, :cw]
                    )
                pa = pp.tile([8, TW], fp32, tag="pa", name="pa")
                for k in range(2):
                    nc.tensor.matmul(
                        pa[:, :cw],
                        w_sb[k][:, 256:264],
                        xt[k][:, :cw],
                        start=(k == 0),
                        stop=(k == 1),
                    )
                oa = pool.tile([8, TW], fp32, tag="oa", name="oa")
                nc.scalar.activation(oa[:, :cw], pa[:, :cw], Act.Copy)
                nc.sync.dma_start(alT[:, c0 : c0 + cw], oa[:, :cw])
    nc.compile()
    return nc


# ------------------------------------------------------------ edge machinery


def _logits_phase(nc, tc, d, L, NCH, ewb):
    """Batched per-edge softmax numerators: ewb = exp(lrelu(as+ad)) from
    hi/lo bf16 pieces, computed up-front for all chunks."""
    NBLK = 2
    nb = (NCH + NBLK - 1) // NBLK
    with tc.tile_pool(name="logit", bufs=2) as pool:
        for b in range(NBLK):
            b0 = b * nb
            bw = min(nb, NCH - b0)
            if bw <= 0:
                break
            lb = pool.tile([128, nb, 16], bf16, tag="lb", name="lb")
            nc.sync.dma_start(lb[:, :bw, :], L[:, b0 : b0 + bw, :])
            e8 = pool.tile([128, nb, 8], fp32, tag="e8", name="e8")
            nc.vector.tensor_tensor(
                e8[:, :bw, :], lb[:, :bw, 0:8], lb[:, :bw, 8:16], op=Alu.add
            )
            e4 = pool.tile([128, nb, 4], fp32, tag="e4", name="e4")
            nc.vector.tensor_tensor(
                e4[:, :bw, :], e8[:, :bw, 0:4], e8[:, :bw, 4:8], op=Alu.add
            )
            nc.vector.scalar_tensor_tensor(
                e4[:, :bw, :],
                e4[:, :bw, :],
                NEG_SLOPE,
                e4[:, :bw, :],
                op0=Alu.mult,
                op1=Alu.max,
            )
            nc.scalar.activation(ewb[:, b0 : b0 + bw, :], e4[:, :bw, :], Act.Exp)


def _edge_pass(nc, tc, d, P, Ks, ewb, rr_sb, iota_sb, fin, pp):
    """Stream pre-expanded 260-wide payload rows (4 blocks x (64 feats +
    w-slot)), weight by ewb, one-hot segment-sum into per-tile PSUM."""
    NCH = sum(Ks)

    with tc.tile_pool(name="edge", bufs=3) as pool:
        state = dict(ncalls=0, tiles={})

        def emit_call(call):
            c0 = call * CPC
            nch = min(CPC, NCH - c0)
            G = pool.tile([128, CPC, 264], bf16, tag="G", name="G", bufs=6)
            OH = pool.tile([128, CPC, 128], bf16, tag="OH", name="OH", bufs=6)
            eng = nc.sync if call % 2 == 0 else nc.scalar
            eng.dma_start(G[:, :nch, :], P[:, c0 : c0 + nch, :])
            rb = rr_sb[:, c0 : c0 + nch].unsqueeze(2).broadcast_to([128, nch, 128])
            # in0 must be REAL packed data for the fast DVE path (measured:
            # any broadcast on port 0 runs ~4x slower); iota_sb is the iota
            # row materialized CPC times
            nc.vector.tensor_tensor(OH[:, :nch, :], iota_sb[:, :nch, :], rb, op=Alu.is_equal)
            g4 = G[:, :nch, :].rearrange("p c (h e) -> p c h e", e=66)
            wb = (
                ewb[:, c0 : c0 + nch, :]
                .unsqueeze(3)
                .broadcast_to([128, nch, 4, 66])
            )
            # payload w-slots are 1.0 from the host, so this multiply also
            # writes the per-block softmax-denominator columns
            nc.vector.tensor_tensor(g4, g4, wb, op=Alu.mult)
            return G, OH

        c = 0
        for s in range(len(Ks)):
            ps = pp.tile([128, 264], fp32, tag="ps", name="ps", bufs=4)
            for k in range(Ks[s]):
                call, cin = c // CPC, c % CPC
                if call >= state["ncalls"]:
                    state["tiles"][call] = emit_call(call)
                    state["ncalls"] = call + 1
                    state["tiles"].pop(call - 5, None)
                G, OH = state["tiles"][call]
                nc.tensor.matmul(
                    ps[:],
                    OH[:, cin, :],
                    G[:, cin, :],
                    start=(k == 0),
                    stop=(k == Ks[s] - 1),
                )
                c += 1
            fin(s, ps)


# ---------------------------------------------------------------- launch 2


def build_l2(d, Ks):
    """Layer-0 edge pass (softmax-div + bias + ELU fused in finalize),
    then table1^T = W1e^T @ h0'^T via a DMA-transpose round trip."""
    nc = bacc.Bacc(None, target_bir_lowering=False, debug=False)
    NP, NT, H = d["NP"], d["NT"], d["H"]
    NCH = sum(Ks)

    P = nc.dram_tensor("P", [128, NCH, 264], bf16, kind="ExternalInput")
    L = nc.dram_tensor("L", [128, NCH, 16], bf16, kind="ExternalInput")
    RR = nc.dram_tensor("RR", [128, NCH], bf16, kind="ExternalInput")
    IOTA = nc.dram_tensor("IOTA", [128, CPC * 128], bf16, kind="ExternalInput")
    W1e = nc.dram_tensor("W1e", [256, 66], bf16, kind="ExternalInput")
    B0 = nc.dram_tensor("B0", [128, 256], bf16, kind="ExternalInput")
    t1T = nc.dram_tensor("t1T", [64, NP], bf16, kind="ExternalOutput")
    a1T = nc.dram_tensor("a1T", [2, NP], fp32, kind="ExternalOutput")

    with tile.TileContext(nc) as tc:
        with (
            tc.tile_pool(name="const", bufs=1) as cpool,
            tc.tile_pool(name="persist", bufs=1) as ipool,
            tc.tile_pool(name="fin", bufs=3) as fpool,
            tc.tile_pool(name="psum", bufs=1, space="PSUM") as pp,
        ):
            iota_sb = cpool.tile([128, CPC, 128], bf16)
            nc.sync.dma_start(iota_sb[:], IOTA[:].rearrange("p (c i) -> p c i", i=128))
            b0_sb = cpool.tile([128, 256], bf16)
            nc.sync.dma_start(b0_sb[:], B0[:])
            rr_sb = ipool.tile([128, NCH], bf16)
            nc.sync.dma_start(rr_sb[:], RR[:])
            ewb = ipool.tile([128, NCH, 4], bf16)
            H0 = ipool.tile([128, NT, 256], bf16)

            _logits_phase(nc, tc, d, L, NCH, ewb)

            def fin0(s, ps):
                sb = fpool.tile([128, 264], fp32, tag="sb", name="sb")
                nc.scalar.activation(sb[:], ps[:], Act.Copy)
                pv = sb[:].rearrange("p (h e) -> p h e", h=H)
                dn = fpool.tile([128, H], fp32, tag="dn", name="dn")
                nc.vector.tensor_scalar_add(dn[:], pv[:, :, 64], EPS)
                rec = fpool.tile([128, H], fp32, tag="rec", name="rec")
                nc.vector.reciprocal(rec[:], dn[:])
                xp = fpool.tile([128, 256], bf16, tag="xp", name="xp")
                rb = rec[:].unsqueeze(2).broadcast_to([128, H, 64])
                nc.vector.tensor_tensor(
                    xp[:].rearrange("p (h e) -> p h e", h=H),
                    pv[:, :, 0:64],
                    rb,
                    op=Alu.mult,
                )
                z = fpool.tile([128, 256], bf16, tag="z", name="z")
                nc.vector.tensor_tensor(z[:], xp[:], b0_sb[:], op=Alu.add)
                ex = fpool.tile([128, 256], fp32, tag="ex", name="ex")
                nc.scalar.activation(ex[:], z[:], Act.Exp, bias=1.0)
                m1 = fpool.tile([128, 256], fp32, tag="m1", name="m1")
                nc.vector.tensor_scalar_min(m1[:], ex[:], 1.0)
                nc.vector.scalar_tensor_tensor(
                    H0[:, s, :], z[:], -1.0, m1[:], op0=Alu.max, op1=Alu.add
                )

            _edge_pass(nc, tc, d, P, Ks, ewb, rr_sb, iota_sb, fin0, pp)

            with (
                tc.tile_pool(name="tb1", bufs=3) as tpool,
                tc.tile_pool(name="dram", bufs=1, space="DRAM") as dpool,
                tc.tile_pool(name="tb1psum", bufs=3, space="PSUM") as pp1,
            ):
                h0d = dpool.tile([NP, 256], bf16)
                nc.sync.dma_start(
                    h0d[:].rearrange("(t p) f -> p t f", p=128), H0[:, :, :]
                )
                h0T = [
                    ipool.tile([128, NP], bf16, tag=f"h0T{k}", name=f"h0T{k}")
                    for k in range(2)
                ]
                for k in range(2):
                    nc.sync.dma_start_transpose(
                        h0T[k][:], h0d[:, 128 * k : 128 * (k + 1)]
                    )
                w1_sb = [
                    cpool.tile([128, 66], bf16, tag=f"w1_{k}", name=f"w1_{k}")
                    for k in range(2)
                ]
                for k in range(2):
                    nc.sync.dma_start(w1_sb[k][:], W1e[128 * k : 128 * (k + 1), :])
                TW = 512
                for j in range((NP + TW - 1) // TW):
                    c0 = j * TW
                    cw = min(TW, NP - c0)
                    pt = pp1.tile([66, TW], fp32, tag="pt", name="pt")
                    for k in range(2):
                        nc.tensor.matmul(
                            pt[:, :cw],
                            w1_sb[k][:],
                            h0T[k][:, c0 : c0 + cw],
                            start=(k == 0),
                            stop=(k == 1),
                        )
                    tb = tpool.tile([64, TW], bf16, tag="tb", name="tb")
                    nc.scalar.activation(tb[:, :cw], pt[0:64, :cw], Act.Copy)
                    nc.sync.dma_start(t1T[:, c0 : c0 + cw], tb[:, :cw])
                    ab = tpool.tile([2, TW], fp32, tag="ab", name="ab")
                    nc.scalar.activation(ab[:, :cw], pt[64:66, :cw], Act.Copy)
                    nc.sync.dma_start(a1T[:, c0 : c0 + cw], ab[:, :cw])
    nc.compile()
    return nc


# ---------------------------------------------------------------- launch 3


def build_l3(d, Ks):
    """Layer-1 edge pass, quad-packed (4 same-dst edges per 260-wide row);
    finalize = sum quads, softmax-div, bias."""
    nc = bacc.Bacc(None, target_bir_lowering=False, debug=False)
    NP, C = d["NP"], d["C_OUT"]
    NCH = sum(Ks)

    P = nc.dram_tensor("P", [128, NCH, 264], bf16, kind="ExternalInput")
    L = nc.dram_tensor("L", [128, NCH, 16], bf16, kind="ExternalInput")
    RR = nc.dram_tensor("RR", [128, NCH], bf16, kind="ExternalInput")
    IOTA = nc.dram_tensor("IOTA", [128, CPC * 128], bf16, kind="ExternalInput")
    B1 = nc.dram_tensor("B1", [128, C], fp32, kind="ExternalInput")
    out = nc.dram_tensor("out", [NP, C], fp32, kind="ExternalOutput")

    with tile.TileContext(nc) as tc:
        with (
            tc.tile_pool(name="const", bufs=1) as cpool,
            tc.tile_pool(name="persist", bufs=1) as ipool,
            tc.tile_pool(name="fin", bufs=3) as fpool,
            tc.tile_pool(name="psum", bufs=1, space="PSUM") as pp,
        ):
            iota_sb = cpool.tile([128, CPC, 128], bf16)
            nc.sync.dma_start(iota_sb[:], IOTA[:].rearrange("p (c i) -> p c i", i=128))
            b1_sb = cpool.tile([128, C], fp32)
            nc.sync.dma_start(b1_sb[:], B1[:])
            rr_sb = ipool.tile([128, NCH], bf16)
            nc.sync.dma_start(rr_sb[:], RR[:])
            ewb = ipool.tile([128, NCH, 4], bf16)

            _logits_phase(nc, tc, d, L, NCH, ewb)

            def fin1(s, ps):
                sb = fpool.tile([128, 264], fp32, tag="sb", name="sb")
                nc.scalar.activation(sb[:], ps[:], Act.Copy)
                sv = sb[:].rearrange("p (q e) -> p q e", q=4)
                a01 = fpool.tile([128, 66], fp32, tag="a01", name="a01")
                nc.vector.tensor_tensor(a01[:], sv[:, 0, :], sv[:, 1, :], op=Alu.add)
                a23 = fpool.tile([128, 66], fp32, tag="a23", name="a23")
                nc.vector.tensor_tensor(a23[:], sv[:, 2, :], sv[:, 3, :], op=Alu.add)
                tot = fpool.tile([128, 66], fp32, tag="tot", name="tot")
                nc.vector.tensor_tensor(tot[:], a01[:], a23[:], op=Alu.add)
                dn = fpool.tile([128, 1], fp32, tag="dnq", name="dnq")
                nc.vector.tensor_scalar_add(dn[:], tot[:, 64:65], EPS)
                rec = fpool.tile([128, 1], fp32, tag="recq", name="recq")
                nc.vector.reciprocal(rec[:], dn[:])
                O = fpool.tile([128, C], fp32, tag="O", name="O")
                nc.vector.scalar_tensor_tensor(
                    O[:], tot[:, 0:64], rec[:], b1_sb[:], op0=Alu.mult, op1=Alu.add
                )
                nc.sync.dma_start(out[128 * s : 128 * (s + 1), :], O[:])

            _edge_pass(nc, tc, d, P, Ks, ewb, rr_sb, iota_sb, fin1, pp)
    nc.compile()
    return nc


# ------------------------------------------------------------ host plumbing


def _bf16(a):
    import ml_dtypes

    return np.asarray(a).astype(ml_dtypes.bfloat16)


def _hilo(a):
    """fp32 array -> (hi, lo) bf16 with hi+lo ~= a."""
    hi = _bf16(a)
    lo = _bf16(a - hi.astype(np.float32))
    return hi, lo


def _build_A0(att_src, att_dst):
    H, DH = att_src.shape
    A = np.zeros((H * DH, 2 * H), np.float32)
    for h in range(H):
        A[h * DH : (h + 1) * DH, h] = att_src[h]
        A[h * DH : (h + 1) * DH, H + h] = att_dst[h]
    return A


def _prep_edges(edge_index, d):
    """Per-core slot structure for both layers.

    l2 (per-edge): slots = dst tiles sorted by edge count (desc) per core;
    K2[s] = max over cores of ceil(count/128).
    l3 (quad): 4 same-dst edges per row; slots = tiles sorted by quad
    count. Returns per-core index arrays into the node tables.
    """
    N, NLOC, NT = d["N"], d["NLOC"], d["NT"]
    src = np.concatenate([edge_index[0], np.arange(N, dtype=np.int64)])
    dst = np.concatenate([edge_index[1], np.arange(N, dtype=np.int64)])
    core = dst // NLOC

    percore = []
    for c in range(NCORES):
        m = core == c
        s_c, t_c = src[m], dst[m] - c * NLOC
        order = np.argsort(t_c, kind="stable")
        percore.append((s_c[order], t_c[order]))

    # ---- layer-0 structure (per edge)
    counts2 = np.zeros((NCORES, NT), np.int64)
    for c in range(NCORES):
        counts2[c] = np.bincount(percore[c][1] // 128, minlength=NT)
    perm2 = np.argsort(-counts2, axis=1, kind="stable")  # [core, slot] -> tile
    sorted2 = -np.sort(-counts2, axis=1)
    K2 = tuple(int(k) for k in np.ceil(sorted2.max(axis=0) / 128).astype(int))
    NCH2 = sum(K2)
    base2 = np.concatenate([[0], np.cumsum(np.array(K2) * 128)])

    l2 = []
    for c in range(NCORES):
        s_c, t_c = percore[c]
        tile_of = t_c // 128
        EP = NCH2 * 128
        gsrc = np.zeros(EP, np.int64)
        gdst = np.zeros(EP, np.int64)
        rr = np.full(EP, -1.0, np.float32)
        pad = np.ones(EP, bool)
        offs = np.concatenate([[0], np.cumsum(counts2[c][perm2[c]])])
        # edges are tile-sorted; index ranges per tile:
        tstart = np.concatenate([[0], np.cumsum(counts2[c])])
        for s in range(NT):
            tl = perm2[c][s]
            n = counts2[c][tl]
            sl = slice(tstart[tl], tstart[tl] + n)
            b = base2[s]
            gsrc[b : b + n] = s_c[sl]
            gdst[b : b + n] = t_c[sl] + c * NLOC
            rr[b : b + n] = (t_c[sl] - 128 * tl).astype(np.float32)
            pad[b : b + n] = False
        l2.append(dict(gsrc=gsrc, gdst=gdst, rr=rr, pad=pad))

    # ---- layer-1 structure (quads)
    counts3 = np.zeros((NCORES, NT), np.int64)
    quads_pc = []
    for c in range(NCORES):
        s_c, t_c = percore[c]
        deg = np.bincount(t_c, minlength=NLOC)
        nq = (deg + 3) // 4  # quads per node
        counts3[c] = np.add.reduceat(
            nq, np.arange(0, NLOC, 128)
        )
        quads_pc.append((s_c, t_c, deg, nq))
    perm3 = np.argsort(-counts3, axis=1, kind="stable")
    sorted3 = -np.sort(-counts3, axis=1)
    K3 = tuple(int(k) for k in np.ceil(sorted3.max(axis=0) / 128).astype(int))
    NCH3 = sum(K3)
    base3 = np.concatenate([[0], np.cumsum(np.array(K3) * 128)])

    l3 = []
    for c in range(NCORES):
        s_c, t_c, deg, nq = quads_pc[c]
        EP = NCH3 * 128
        qsrc = np.zeros((EP, 4), np.int64)
        qdst = np.zeros(EP, np.int64)
        rr = np.full(EP, -1.0, np.float32)
        pad = np.ones((EP, 4), bool)
        estart = np.concatenate([[0], np.cumsum(deg)])
        qstart_tile = np.concatenate(
            [[0], np.cumsum(counts3[c])]
        )  # quad offset per tile (in tile order)
        for s in range(NT):
            tl = perm3[c][s]
            b = base3[s]
            q = 0
            n0 = tl * 128
            n1 = min(n0 + 128, NLOC)
            for node in range(n0, n1):
                dg = deg[node]
                if dg == 0:
                    continue
                e0 = estart[node]
                nqn = nq[node]
                rows = b + q + np.arange(nqn)
                rr[rows] = float(node - n0)
                qdst[rows] = node + c * NLOC
                es = s_c[e0 : e0 + dg]
                full = np.zeros(nqn * 4, np.int64)
                full[:dg] = es
                qsrc[rows] = full.reshape(nqn, 4)
                pd = np.ones(nqn * 4, bool)
                pd[:dg] = False
                pad[rows] = pd.reshape(nqn, 4)
                q += nqn
        l3.append(dict(qsrc=qsrc, qdst=qdst, rr=rr, pad=pad))

    return dict(K2=K2, K3=K3, perm2=perm2, perm3=perm3, l2=l2, l3=l3)


def _pack_pm(a, nch):
    """[EP, W] row-major -> [128, nch, W] partition-major contiguous."""
    W = a.shape[1]
    return np.ascontiguousarray(a.reshape(nch, 128, W).transpose(1, 0, 2))


def _expand_l2(core_idx, tab0, a0, prep):
    """Per-core launch-2 inputs from full node tables (pure gather)."""
    K2 = prep["K2"]
    NCH = sum(K2)
    e = prep["l2"][core_idx]
    gsrc, gdst, pad = e["gsrc"], e["gdst"], e["pad"]
    EP = NCH * 128
    rows = tab0[gsrc]  # [EP, 256] bf16
    P = np.zeros((EP, 264), rows.dtype)
    pv = P.reshape(EP, 4, 66)
    pv[:, :, 0:64] = rows.reshape(EP, 4, 64)
    pv[:, :, 64] = 1.0  # weighting writes w into these denominator slots
    as_hi, as_lo = a0["as_hi"][gsrc], a0["as_lo"][gsrc]
    ad_hi, ad_lo = a0["ad_hi"][gdst], a0["ad_lo"][gdst]
    L = np.concatenate([as_hi, as_lo, ad_hi, ad_lo], axis=1)
    L[pad, 0:4] = PAD_LOGIT
    rr = e["rr"].reshape(NCH, 128).T
    return dict(
        P=_pack_pm(P, NCH),
        L=_pack_pm(L, NCH),
        RR=np.ascontiguousarray(_bf16(rr)),
    )


def _expand_l3(core_idx, tab1, a1, prep):
    K3 = prep["K3"]
    NCH = sum(K3)
    e = prep["l3"][core_idx]
    qsrc, qdst, pad = e["qsrc"], e["qdst"], e["pad"]
    EP = NCH * 128
    P = np.zeros((EP, 264), tab1.dtype)
    pv = P.reshape(EP, 4, 66)
    for j in range(4):
        pv[:, j, 0:64] = tab1[qsrc[:, j]]
    pv[:, :, 64] = 1.0  # weighting writes w into these denominator slots
    as_hi = a1["as_hi"][qsrc]  # [EP, 4]
    as_lo = a1["as_lo"][qsrc]
    ad_hi = np.repeat(a1["ad_hi"][qdst][:, None], 4, axis=1)
    ad_lo = np.repeat(a1["ad_lo"][qdst][:, None], 4, axis=1)
    L = np.concatenate([as_hi, as_lo, ad_hi, ad_lo], axis=1)
    L[:, 0:4][pad] = PAD_LOGIT
    rr = e["rr"].reshape(NCH, 128).T
    return dict(
        P=_pack_pm(P, NCH),
        L=_pack_pm(L, NCH),
        RR=np.ascontiguousarray(_bf16(rr)),
    )


_cache = {}
LAST_PROFILE = {}


def _run(nc, in_maps, core_ids, label):
    trace = bool(int(os.environ.get("GAT_PROFILE", "0")))
    if trace:
        try:
            import sys

            import profile_hook

            profile_hook.install()
            import concourse.bass_utils as bu

            bu.upload_artifacts = lambda tmpdir: "local://skipped"
            tdir = f"/tmp/gat_trace_{label}"
            os.makedirs(tdir, exist_ok=True)
            for f in os.listdir(tdir):
                os.unlink(os.path.join(tdir, f))
            br = run_bass_kernel_spmd(nc, in_maps, core_ids, trace=True, tmpdir=tdir)
            LAST_PROFILE[label] = br.exec_time_ns
            return br.results
        except Exception as e:  # fall back to untraced
            print(f"traced run failed ({e!r}); untraced retry", file=sys.stderr)
    br = run_bass_kernel_spmd(nc, in_maps, core_ids)
    LAST_PROFILE[label] = br.exec_time_ns
    return br.results


def kernel(x, edge_index, W0, att_src0, att_dst0, b0, W1, att_src1, att_dst1, b1):
    x = np.asarray(x, np.float32)
    edge_index = np.asarray(edge_index)
    d = _dims()
    N, NLOC, NP, NT = d["N"], d["NLOC"], d["NP"], d["NT"]

    prep = _prep_edges(edge_index, d)
    key = (prep["K2"], prep["K3"])
    if key not in _cache:
        _cache[key] = (build_l1(d), build_l2(d, prep["K2"]), build_l3(d, prep["K3"]))
    nc1, nc2, nc3 = _cache[key]

    A0 = _build_A0(np.asarray(att_src0), np.asarray(att_dst0))
    W0f = np.asarray(W0, np.float32)
    W0e = _bf16(np.concatenate([W0f, W0f @ A0], axis=1))
    W1f = np.asarray(W1, np.float32)
    was1 = W1f @ np.asarray(att_src1, np.float32).ravel()
    wad1 = W1f @ np.asarray(att_dst1, np.float32).ravel()
    W1e = _bf16(np.stack([*W1f.T, was1, wad1], axis=1))  # [256, 66]
    b0m1 = np.tile(np.asarray(b0, np.float32)[None, :] - 1.0, (128, 1))
    b1r = np.tile(np.asarray(b1, np.float32)[None, :], (128, 1))
    iota = _bf16(np.tile(np.arange(128, dtype=np.float32)[None, :], (128, CPC)))
    core_ids = list(range(NCORES))

    # launch 1
    xb = _bf16(x)
    in1 = []
    for c in range(NCORES):
        xT = np.zeros((d["F_IN"], NP), xb.dtype)
        xT[:, :NLOC] = xb[c * NLOC : (c + 1) * NLOC].T
        in1.append(dict(xT=xT, W0e=W0e))
    r1 = _run(nc1, in1, core_ids, "l1")

    tab0 = np.ascontiguousarray(
        np.concatenate(
            [r1[c]["t0T"][:, :NLOC] for c in range(NCORES)], axis=1
        ).T
    )  # [N, 256] bf16
    alf = np.concatenate([r1[c]["alT"][:, :NLOC] for c in range(NCORES)], axis=1)
    as_hi, as_lo = _hilo(alf[0:4].T)
    ad_hi, ad_lo = _hilo(alf[4:8].T)
    a0 = dict(as_hi=as_hi, as_lo=as_lo, ad_hi=ad_hi, ad_lo=ad_lo)

    in2 = [
        dict(
            _expand_l2(c, tab0, a0, prep),
            IOTA=iota,
            W1e=W1e,
            B0=_bf16(b0m1),
        )
        for c in range(NCORES)
    ]
    r2 = _run(nc2, in2, core_ids, "l2")

    # un-permute slot-major table1 columns -> node order
    tab1 = np.zeros((N, 64), r2[0]["t1T"].dtype)
    a1sh = np.zeros(N, np.float32)
    a1dh = np.zeros(N, np.float32)
    for c in range(NCORES):
        t1 = r2[c]["t1T"]  # [64, NP] slot-major
        a1c = r2[c]["a1T"]  # [2, NP]
        for s in range(NT):
            tl = prep["perm2"][c][s]
            n0 = tl * 128
            n1 = min(n0 + 128, NLOC)
            w = n1 - n0
            if w <= 0:
                continue
            tab1[c * NLOC + n0 : c * NLOC + n1] = t1[:, 128 * s : 128 * s + w].T
            a1sh[c * NLOC + n0 : c * NLOC + n1] = a1c[0, 128 * s : 128 * s + w]
            a1dh[c * NLOC + n0 : c * NLOC + n1] = a1c[1, 128 * s : 128 * s + w]
    s_hi, s_lo = _hilo(a1sh)
    d_hi, d_lo = _hilo(a1dh)
    a1 = dict(as_hi=s_hi, as_lo=s_lo, ad_hi=d_hi, ad_lo=d_lo)

    in3 = [
        dict(_expand_l3(c, tab1, a1, prep), IOTA=iota, B1=b1r)
        for c in range(NCORES)
    ]
    r3 = _run(nc3, in3, core_ids, "l3")

    out = np.zeros((N, 64), np.float32)
    for c in range(NCORES):
        o = r3[c]["out"]  # [NP, 64] slot-major
        for s in range(NT):
            tl = prep["perm3"][c][s]
            n0 = tl * 128
            n1 = min(n0 + 128, NLOC)
            w = n1 - n0
            if w <= 0:
                continue
            out[c * NLOC + n0 : c * NLOC + n1] = o[128 * s : 128 * s + w]
    return out


# revision 26
# speedup vs baseline: 1.4402x; 1.2254x over previous
"""Two-layer GAT (PyG-style GATConv x2) on 8 Trainium2 NeuronCores.

Design (v2, "host-expand"): nodes are sharded across the 8 cores by
destination. Between launches the HOST rearranges device-computed tables
(pure data movement: fancy-indexed row expansion per edge, sorting,
padding, hi/lo bf16 splits). All model arithmetic (matmuls, logit
add/leaky-relu/exp, softmax division, weighting, ELU, bias) runs on
device.

Rationale: per-edge SWDGE dma_gather costs ~8.3ns/edge of *serial* Q7
descriptor generation (~0.9ms/layer/core) - the measured bottleneck of
the v1 kernel. Pre-expanding edge payload rows on the host turns the
edge pass into dense sequential DMA + one-hot segment-sum matmuls.

Per-edge layout: edges are sorted by dst tile; each dst tile's edges are
padded to a multiple of 128 ("chunks"). Tiles are assigned to "slots" in
decreasing-count order per core so chunk counts align across the 8 SPMD
cores with minimal padding (the host un-permutes outputs).

Layer 1 packs 4 edges of the same dst node into one 260-wide row
(4 x (64 feats + w)), quartering the one-hot matmul count.

Three SPMD launches with host-side expansion between them:
  1. table0: h0^T = W0e^T @ x^T  -> feat-major table + per-node alphas
  2. layer-0 edges: stream payload/softmax/one-hot matmul -> ELU ->
     h1 = h0' @ W1e -> table1 (feat-major) + alphas
  3. layer-1 edges (quad-packed): same -> bias -> output shard
"""

import os

import numpy as np

import concourse.bacc as bacc
import concourse.mybir as mybir
from concourse import tile
from concourse.bass_utils import run_bass_kernel_spmd

fp32 = mybir.dt.float32
bf16 = mybir.dt.bfloat16
Alu = mybir.AluOpType
Act = mybir.ActivationFunctionType

NCORES = 8
NEG_SLOPE = 0.2
EPS = 1e-16
PAD_LOGIT = -30000.0
CPC = 16  # chunks per payload DMA call


def _dims():
    return dict(
        N=50000,
        NLOC=6250,
        NP=6272,  # padded to mult of 128
        NT=49,
        F_IN=256,
        HID=256,
        H=4,
        DH=64,
        C_OUT=64,
    )


# ---------------------------------------------------------------- launch 1


def build_l1(d):
    """h0^T = W0e^T @ x^T per core; W0e = [W0 | W0@A0] folds the per-node
    attention alphas into the same matmul. Outputs feat-major bf16 table
    plus fp32 alphas (host splits hi/lo)."""
    nc = bacc.Bacc(None, target_bir_lowering=False, debug=False)
    NP, F = d["NP"], d["F_IN"]

    xT = nc.dram_tensor("xT", [F, NP], bf16, kind="ExternalInput")
    W0e = nc.dram_tensor("W0e", [F, 264], bf16, kind="ExternalInput")
    t0T = nc.dram_tensor("t0T", [256, NP], bf16, kind="ExternalOutput")
    alT = nc.dram_tensor("alT", [8, NP], fp32, kind="ExternalOutput")

    TW = 512
    n_t = (NP + TW - 1) // TW

    with tile.TileContext(nc) as tc:
        with (
            tc.tile_pool(name="const", bufs=1) as cpool,
            tc.tile_pool(name="work", bufs=3) as pool,
            tc.tile_pool(name="psum", bufs=2, space="PSUM") as pp,
        ):
            w_sb = [
                cpool.tile([128, 264], bf16, tag=f"w{k}", name=f"w{k}")
                for k in range(2)
            ]
            for k in range(2):
                nc.sync.dma_start(w_sb[k][:], W0e[128 * k : 128 * (k + 1), :])

            for t in range(n_t):
                c0 = t * TW
                cw = min(TW, NP - c0)
                xt = [
                    pool.tile([128, TW], bf16, tag=f"xt{k}", name=f"xt{k}")
                    for k in range(2)
                ]
                for k in range(2):
                    nc.sync.dma_start(
                        xt[k][:, :cw], xT[128 * k : 128 * (k + 1), c0 : c0 + cw]
                    )
                for m in range(2):
                    ps = pp.tile([128, TW], fp32, tag=f"ps{m}", name=f"ps{m}")
                    for k in range(2):
                        nc.tensor.matmul(
                            ps[:, :cw],
                            w_sb[k][:, 128 * m : 128 * (m + 1)],
                            xt[k][:, :cw],
                            start=(k == 0),
                            stop=(k == 1),
                        )
                    ob = pool.tile([128, TW], bf16, tag=f"ob{m}", name=f"ob{m}")
                    nc.scalar.activation(ob[:, :cw], ps[:, :cw], Act.Copy)
                    nc.sync.dma_start(
                        t0T[128 * m : 128 * (m + 1), c0 : c0 + cw], ob[:, :cw]
                    )
                pa = pp.tile([8, TW], fp32, tag="pa", name="pa")
                for k in range(2):
                    nc.tensor.matmul(
                        pa[:, :cw],
                        w_sb[k][:, 256:264],
                        xt[k][:, :cw],
                        start=(k == 0),
                        stop=(k == 1),
                    )
                oa = pool.tile([8, TW], fp32, tag="oa", name="oa")
                nc.scalar.activation(oa[:, :cw], pa[:, :cw], Act.Copy)
                nc.sync.dma_start(alT[:, c0 : c0 + cw], oa[:, :cw])
    nc.compile()
    return nc


# ------------------------------------------------------------ edge machinery


def _logits_phase(nc, tc, d, L, NCH, ewb):
    """Batched per-edge softmax numerators: ewb = exp(lrelu(as+ad)) from
    hi/lo bf16 pieces, computed up-front for all chunks."""
    NBLK = 2
    nb = (NCH + NBLK - 1) // NBLK
    with tc.tile_pool(name="logit", bufs=2) as pool:
        for b in range(NBLK):
            b0 = b * nb
            bw = min(nb, NCH - b0)
            if bw <= 0:
                break
            lb = pool.tile([128, nb, 16], bf16, tag="lb", name="lb")
            nc.sync.dma_start(lb[:, :bw, :], L[:, b0 : b0 + bw, :])
            e8 = pool.tile([128, nb, 8], fp32, tag="e8", name="e8")
            nc.vector.tensor_tensor(
                e8[:, :bw, :], lb[:, :bw, 0:8], lb[:, :bw, 8:16], op=Alu.add
            )
            e4 = pool.tile([128, nb, 4], fp32, tag="e4", name="e4")
            nc.vector.tensor_tensor(
                e4[:, :bw, :], e8[:, :bw, 0:4], e8[:, :bw, 4:8], op=Alu.add
            )
            nc.vector.scalar_tensor_tensor(
                e4[:, :bw, :],
                e4[:, :bw, :],
                NEG_SLOPE,
                e4[:, :bw, :],
                op0=Alu.mult,
                op1=Alu.max,
            )
            nc.scalar.activation(ewb[:, b0 : b0 + bw, :], e4[:, :bw, :], Act.Exp)


def _edge_pass(nc, tc, d, P, OHD, Ks, ewb, fin, pp):
    """Stream pre-expanded 260-wide payload rows (4 blocks x (64 feats +
    w-slot)), weight by ewb, one-hot segment-sum into per-tile PSUM."""
    NCH = sum(Ks)

    with tc.tile_pool(name="edge", bufs=3) as pool:
        state = dict(ncalls=0, tiles={})

        def emit_call(call):
            c0 = call * CPC
            nch = min(CPC, NCH - c0)
            G = pool.tile([128, CPC, 264], bf16, tag="G", name="G", bufs=6)
            OH = pool.tile([128, CPC, 128], bf16, tag="OH", name="OH", bufs=6)
            # one-hots are host-built and streamed: a DVE is_equal build
            # measures ~2.2ns/elem (no fast uop + broadcast-port penalty),
            # so DMA is the cheaper engine for them
            nc.sync.dma_start(G[:, :nch, :], P[:, c0 : c0 + nch, :])
            nc.scalar.dma_start(OH[:, :nch, :], OHD[:, c0 : c0 + nch, :])
            g4 = G[:, :nch, :].rearrange("p c (h e) -> p c h e", e=66)
            wb = (
                ewb[:, c0 : c0 + nch, :]
                .unsqueeze(3)
                .broadcast_to([128, nch, 4, 66])
            )
            # payload w-slots are 1.0 from the host, so this multiply also
            # writes the per-block softmax-denominator columns
            nc.vector.tensor_tensor(g4, g4, wb, op=Alu.mult)
            return G, OH

        c = 0
        for s in range(len(Ks)):
            ps = pp.tile([128, 264], fp32, tag="ps", name="ps", bufs=4)
            for k in range(Ks[s]):
                call, cin = c // CPC, c % CPC
                if call >= state["ncalls"]:
                    state["tiles"][call] = emit_call(call)
                    state["ncalls"] = call + 1
                    state["tiles"].pop(call - 5, None)
                G, OH = state["tiles"][call]
                nc.tensor.matmul(
                    ps[:],
                    OH[:, cin, :],
                    G[:, cin, :],
                    start=(k == 0),
                    stop=(k == Ks[s] - 1),
                )
                c += 1
            fin(s, ps)


# ---------------------------------------------------------------- launch 2


def build_l2(d, Ks):
    """Layer-0 edge pass (softmax-div + bias + ELU fused in finalize),
    then table1^T = W1e^T @ h0'^T via a DMA-transpose round trip."""
    nc = bacc.Bacc(None, target_bir_lowering=False, debug=False)
    NP, NT, H = d["NP"], d["NT"], d["H"]
    NCH = sum(Ks)

    P = nc.dram_tensor("P", [128, NCH, 264], bf16, kind="ExternalInput")
    L = nc.dram_tensor("L", [128, NCH, 16], bf16, kind="ExternalInput")
    OHD = nc.dram_tensor("OHD", [128, NCH, 128], bf16, kind="ExternalInput")
    W1e = nc.dram_tensor("W1e", [256, 66], bf16, kind="ExternalInput")
    B0 = nc.dram_tensor("B0", [128, 256], bf16, kind="ExternalInput")
    t1T = nc.dram_tensor("t1T", [64, NP], bf16, kind="ExternalOutput")
    a1T = nc.dram_tensor("a1T", [2, NP], fp32, kind="ExternalOutput")

    with tile.TileContext(nc) as tc:
        with (
            tc.tile_pool(name="const", bufs=1) as cpool,
            tc.tile_pool(name="persist", bufs=1) as ipool,
            tc.tile_pool(name="fin", bufs=3) as fpool,
            tc.tile_pool(name="psum", bufs=1, space="PSUM") as pp,
        ):
            b0_sb = cpool.tile([128, 256], bf16)
            nc.sync.dma_start(b0_sb[:], B0[:])
            ewb = ipool.tile([128, NCH, 4], bf16)
            H0 = ipool.tile([128, NT, 256], bf16)

            _logits_phase(nc, tc, d, L, NCH, ewb)

            def fin0(s, ps):
                sb = fpool.tile([128, 264], fp32, tag="sb", name="sb")
                nc.scalar.activation(sb[:], ps[:], Act.Copy)
                pv = sb[:].rearrange("p (h e) -> p h e", h=H)
                dn = fpool.tile([128, H], fp32, tag="dn", name="dn")
                nc.vector.tensor_scalar_add(dn[:], pv[:, :, 64], EPS)
                rec = fpool.tile([128, H], fp32, tag="rec", name="rec")
                nc.vector.reciprocal(rec[:], dn[:])
                xp = fpool.tile([128, 256], bf16, tag="xp", name="xp")
                rb = rec[:].unsqueeze(2).broadcast_to([128, H, 64])
                nc.vector.tensor_tensor(
                    xp[:].rearrange("p (h e) -> p h e", h=H),
                    pv[:, :, 0:64],
                    rb,
                    op=Alu.mult,
                )
                z = fpool.tile([128, 256], bf16, tag="z", name="z")
                nc.vector.tensor_tensor(z[:], xp[:], b0_sb[:], op=Alu.add)
                ex = fpool.tile([128, 256], fp32, tag="ex", name="ex")
                nc.scalar.activation(ex[:], z[:], Act.Exp, bias=1.0)
                m1 = fpool.tile([128, 256], fp32, tag="m1", name="m1")
                nc.vector.tensor_scalar_min(m1[:], ex[:], 1.0)
                nc.vector.scalar_tensor_tensor(
                    H0[:, s, :], z[:], -1.0, m1[:], op0=Alu.max, op1=Alu.add
                )

            _edge_pass(nc, tc, d, P, OHD, Ks, ewb, fin0, pp)

            with (
                tc.tile_pool(name="tb1", bufs=3) as tpool,
                tc.tile_pool(name="dram", bufs=1, space="DRAM") as dpool,
                tc.tile_pool(name="tb1psum", bufs=3, space="PSUM") as pp1,
            ):
                h0d = dpool.tile([NP, 256], bf16)
                nc.sync.dma_start(
                    h0d[:].rearrange("(t p) f -> p t f", p=128), H0[:, :, :]
                )
                h0T = [
                    ipool.tile([128, NP], bf16, tag=f"h0T{k}", name=f"h0T{k}")
                    for k in range(2)
                ]
                for k in range(2):
                    nc.sync.dma_start_transpose(
                        h0T[k][:], h0d[:, 128 * k : 128 * (k + 1)]
                    )
                w1_sb = [
                    cpool.tile([128, 66], bf16, tag=f"w1_{k}", name=f"w1_{k}")
                    for k in range(2)
                ]
                for k in range(2):
                    nc.sync.dma_start(w1_sb[k][:], W1e[128 * k : 128 * (k + 1), :])
                TW = 512
                for j in range((NP + TW - 1) // TW):
                    c0 = j * TW
                    cw = min(TW, NP - c0)
                    pt = pp1.tile([66, TW], fp32, tag="pt", name="pt")
                    for k in range(2):
                        nc.tensor.matmul(
                            pt[:, :cw],
                            w1_sb[k][:],
                            h0T[k][:, c0 : c0 + cw],
                            start=(k == 0),
                            stop=(k == 1),
                        )
                    tb = tpool.tile([64, TW], bf16, tag="tb", name="tb")
                    nc.scalar.activation(tb[:, :cw], pt[0:64, :cw], Act.Copy)
                    nc.sync.dma_start(t1T[:, c0 : c0 + cw], tb[:, :cw])
                    ab = tpool.tile([2, TW], fp32, tag="ab", name="ab")
                    nc.scalar.activation(ab[:, :cw], pt[64:66, :cw], Act.Copy)
                    nc.sync.dma_start(a1T[:, c0 : c0 + cw], ab[:, :cw])
    nc.compile()
    return nc


# ---------------------------------------------------------------- launch 3


def build_l3(d, Ks):
    """Layer-1 edge pass, quad-packed (4 same-dst edges per 260-wide row);
    finalize = sum quads, softmax-div, bias."""
    nc = bacc.Bacc(None, target_bir_lowering=False, debug=False)
    NP, C = d["NP"], d["C_OUT"]
    NCH = sum(Ks)

    P = nc.dram_tensor("P", [128, NCH, 264], bf16, kind="ExternalInput")
    L = nc.dram_tensor("L", [128, NCH, 16], bf16, kind="ExternalInput")
    OHD = nc.dram_tensor("OHD", [128, NCH, 128], bf16, kind="ExternalInput")
    B1 = nc.dram_tensor("B1", [128, C], fp32, kind="ExternalInput")
    out = nc.dram_tensor("out", [NP, C], fp32, kind="ExternalOutput")

    with tile.TileContext(nc) as tc:
        with (
            tc.tile_pool(name="const", bufs=1) as cpool,
            tc.tile_pool(name="persist", bufs=1) as ipool,
            tc.tile_pool(name="fin", bufs=3) as fpool,
            tc.tile_pool(name="psum", bufs=1, space="PSUM") as pp,
        ):
            b1_sb = cpool.tile([128, C], fp32)
            nc.sync.dma_start(b1_sb[:], B1[:])
            ewb = ipool.tile([128, NCH, 4], bf16)

            _logits_phase(nc, tc, d, L, NCH, ewb)

            def fin1(s, ps):
                sb = fpool.tile([128, 264], fp32, tag="sb", name="sb")
                nc.scalar.activation(sb[:], ps[:], Act.Copy)
                sv = sb[:].rearrange("p (q e) -> p q e", q=4)
                a01 = fpool.tile([128, 66], fp32, tag="a01", name="a01")
                nc.vector.tensor_tensor(a01[:], sv[:, 0, :], sv[:, 1, :], op=Alu.add)
                a23 = fpool.tile([128, 66], fp32, tag="a23", name="a23")
                nc.vector.tensor_tensor(a23[:], sv[:, 2, :], sv[:, 3, :], op=Alu.add)
                tot = fpool.tile([128, 66], fp32, tag="tot", name="tot")
                nc.vector.tensor_tensor(tot[:], a01[:], a23[:], op=Alu.add)
                dn = fpool.tile([128, 1], fp32, tag="dnq", name="dnq")
                nc.vector.tensor_scalar_add(dn[:], tot[:, 64:65], EPS)
                rec = fpool.tile([128, 1], fp32, tag="recq", name="recq")
                nc.vector.reciprocal(rec[:], dn[:])
                O = fpool.tile([128, C], fp32, tag="O", name="O")
                nc.vector.scalar_tensor_tensor(
                    O[:], tot[:, 0:64], rec[:], b1_sb[:], op0=Alu.mult, op1=Alu.add
                )
                nc.sync.dma_start(out[128 * s : 128 * (s + 1), :], O[:])

            _edge_pass(nc, tc, d, P, OHD, Ks, ewb, fin1, pp)
    nc.compile()
    return nc


# ------------------------------------------------------------ host plumbing


def _bf16(a):
    import ml_dtypes

    return np.asarray(a).astype(ml_dtypes.bfloat16)


def _hilo(a):
    """fp32 array -> (hi, lo) bf16 with hi+lo ~= a."""
    hi = _bf16(a)
    lo = _bf16(a - hi.astype(np.float32))
    return hi, lo


def _build_A0(att_src, att_dst):
    H, DH = att_src.shape
    A = np.zeros((H * DH, 2 * H), np.float32)
    for h in range(H):
        A[h * DH : (h + 1) * DH, h] = att_src[h]
        A[h * DH : (h + 1) * DH, H + h] = att_dst[h]
    return A


def _prep_edges(edge_index, d):
    """Per-core slot structure for both layers.

    l2 (per-edge): slots = dst tiles sorted by edge count (desc) per core;
    K2[s] = max over cores of ceil(count/128).
    l3 (quad): 4 same-dst edges per row; slots = tiles sorted by quad
    count. Returns per-core index arrays into the node tables.
    """
    N, NLOC, NT = d["N"], d["NLOC"], d["NT"]
    src = np.concatenate([edge_index[0], np.arange(N, dtype=np.int64)])
    dst = np.concatenate([edge_index[1], np.arange(N, dtype=np.int64)])
    core = dst // NLOC

    percore = []
    for c in range(NCORES):
        m = core == c
        s_c, t_c = src[m], dst[m] - c * NLOC
        order = np.argsort(t_c, kind="stable")
        percore.append((s_c[order], t_c[order]))

    # ---- layer-0 structure (per edge)
    counts2 = np.zeros((NCORES, NT), np.int64)
    for c in range(NCORES):
        counts2[c] = np.bincount(percore[c][1] // 128, minlength=NT)
    perm2 = np.argsort(-counts2, axis=1, kind="stable")  # [core, slot] -> tile
    sorted2 = -np.sort(-counts2, axis=1)
    K2 = tuple(int(k) for k in np.ceil(sorted2.max(axis=0) / 128).astype(int))
    NCH2 = sum(K2)
    base2 = np.concatenate([[0], np.cumsum(np.array(K2) * 128)])

    l2 = []
    for c in range(NCORES):
        s_c, t_c = percore[c]
        tile_of = t_c // 128
        EP = NCH2 * 128
        gsrc = np.zeros(EP, np.int64)
        gdst = np.zeros(EP, np.int64)
        rr = np.full(EP, -1.0, np.float32)
        pad = np.ones(EP, bool)
        offs = np.concatenate([[0], np.cumsum(counts2[c][perm2[c]])])
        # edges are tile-sorted; index ranges per tile:
        tstart = np.concatenate([[0], np.cumsum(counts2[c])])
        for s in range(NT):
            tl = perm2[c][s]
            n = counts2[c][tl]
            sl = slice(tstart[tl], tstart[tl] + n)
            b = base2[s]
            gsrc[b : b + n] = s_c[sl]
            gdst[b : b + n] = t_c[sl] + c * NLOC
            rr[b : b + n] = (t_c[sl] - 128 * tl).astype(np.float32)
            pad[b : b + n] = False
        l2.append(dict(gsrc=gsrc, gdst=gdst, rr=rr, pad=pad))

    # ---- layer-1 structure (quads)
    counts3 = np.zeros((NCORES, NT), np.int64)
    quads_pc = []
    for c in range(NCORES):
        s_c, t_c = percore[c]
        deg = np.bincount(t_c, minlength=NLOC)
        nq = (deg + 3) // 4  # quads per node
        counts3[c] = np.add.reduceat(
            nq, np.arange(0, NLOC, 128)
        )
        quads_pc.append((s_c, t_c, deg, nq))
    perm3 = np.argsort(-counts3, axis=1, kind="stable")
    sorted3 = -np.sort(-counts3, axis=1)
    K3 = tuple(int(k) for k in np.ceil(sorted3.max(axis=0) / 128).astype(int))
    NCH3 = sum(K3)
    base3 = np.concatenate([[0], np.cumsum(np.array(K3) * 128)])

    l3 = []
    for c in range(NCORES):
        s_c, t_c, deg, nq = quads_pc[c]
        EP = NCH3 * 128
        qsrc = np.zeros((EP, 4), np.int64)
        qdst = np.zeros(EP, np.int64)
        rr = np.full(EP, -1.0, np.float32)
        pad = np.ones((EP, 4), bool)
        estart = np.concatenate([[0], np.cumsum(deg)])
        qstart_tile = np.concatenate(
            [[0], np.cumsum(counts3[c])]
        )  # quad offset per tile (in tile order)
        for s in range(NT):
            tl = perm3[c][s]
            b = base3[s]
            q = 0
            n0 = tl * 128
            n1 = min(n0 + 128, NLOC)
            for node in range(n0, n1):
                dg = deg[node]
                if dg == 0:
                    continue
                e0 = estart[node]
                nqn = nq[node]
                rows = b + q + np.arange(nqn)
                rr[rows] = float(node - n0)
                qdst[rows] = node + c * NLOC
                es = s_c[e0 : e0 + dg]
                full = np.zeros(nqn * 4, np.int64)
                full[:dg] = es
                qsrc[rows] = full.reshape(nqn, 4)
                pd = np.ones(nqn * 4, bool)
                pd[:dg] = False
                pad[rows] = pd.reshape(nqn, 4)
                q += nqn
        l3.append(dict(qsrc=qsrc, qdst=qdst, rr=rr, pad=pad))

    return dict(K2=K2, K3=K3, perm2=perm2, perm3=perm3, l2=l2, l3=l3)


_EYEP = None


def _oh_rows(rr):
    """rr [EP] (float, -1 = padding) -> one-hot rows [EP, 128] bf16."""
    global _EYEP
    if _EYEP is None:
        _EYEP = np.zeros((129, 128), np.float32)
        _EYEP[:128] = np.eye(128, dtype=np.float32)
        _EYEP = _bf16(_EYEP)
    idx = rr.astype(np.int64)
    idx[idx < 0] = 128
    return _EYEP[idx]


def _pack_pm(a, nch):
    """[EP, W] row-major -> [128, nch, W] partition-major contiguous."""
    W = a.shape[1]
    return np.ascontiguousarray(a.reshape(nch, 128, W).transpose(1, 0, 2))


def _expand_l2(core_idx, tab0, a0, prep):
    """Per-core launch-2 inputs from full node tables (pure gather)."""
    K2 = prep["K2"]
    NCH = sum(K2)
    e = prep["l2"][core_idx]
    gsrc, gdst, pad = e["gsrc"], e["gdst"], e["pad"]
    EP = NCH * 128
    rows = tab0[gsrc]  # [EP, 256] bf16
    P = np.zeros((EP, 264), rows.dtype)
    pv = P.reshape(EP, 4, 66)
    pv[:, :, 0:64] = rows.reshape(EP, 4, 64)
    pv[:, :, 64] = 1.0  # weighting writes w into these denominator slots
    as_hi, as_lo = a0["as_hi"][gsrc], a0["as_lo"][gsrc]
    ad_hi, ad_lo = a0["ad_hi"][gdst], a0["ad_lo"][gdst]
    L = np.concatenate([as_hi, as_lo, ad_hi, ad_lo], axis=1)
    L[pad, 0:4] = PAD_LOGIT
    return dict(
        P=_pack_pm(P, NCH),
        L=_pack_pm(L, NCH),
        OHD=_pack_pm(_oh_rows(e["rr"]), NCH),
    )


def _expand_l3(core_idx, tab1, a1, prep):
    K3 = prep["K3"]
    NCH = sum(K3)
    e = prep["l3"][core_idx]
    qsrc, qdst, pad = e["qsrc"], e["qdst"], e["pad"]
    EP = NCH * 128
    P = np.zeros((EP, 264), tab1.dtype)
    pv = P.reshape(EP, 4, 66)
    for j in range(4):
        pv[:, j, 0:64] = tab1[qsrc[:, j]]
    pv[:, :, 64] = 1.0  # weighting writes w into these denominator slots
    as_hi = a1["as_hi"][qsrc]  # [EP, 4]
    as_lo = a1["as_lo"][qsrc]
    ad_hi = np.repeat(a1["ad_hi"][qdst][:, None], 4, axis=1)
    ad_lo = np.repeat(a1["ad_lo"][qdst][:, None], 4, axis=1)
    L = np.concatenate([as_hi, as_lo, ad_hi, ad_lo], axis=1)
    L[:, 0:4][pad] = PAD_LOGIT
    return dict(
        P=_pack_pm(P, NCH),
        L=_pack_pm(L, NCH),
        OHD=_pack_pm(_oh_rows(e["rr"]), NCH),
    )


_cache = {}
LAST_PROFILE = {}


def _run(nc, in_maps, core_ids, label):
    trace = bool(int(os.environ.get("GAT_PROFILE", "0")))
    if trace:
        try:
            import sys

            import profile_hook

            profile_hook.install()
            import concourse.bass_utils as bu

            bu.upload_artifacts = lambda tmpdir: "local://skipped"
            tdir = f"/tmp/gat_trace_{label}"
            os.makedirs(tdir, exist_ok=True)
            for f in os.listdir(tdir):
                os.unlink(os.path.join(tdir, f))
            br = run_bass_kernel_spmd(nc, in_maps, core_ids, trace=True, tmpdir=tdir)
            LAST_PROFILE[label] = br.exec_time_ns
            return br.results
        except Exception as e:  # fall back to untraced
            print(f"traced run failed ({e!r}); untraced retry", file=sys.stderr)
    br = run_bass_kernel_spmd(nc, in_maps, core_ids)
    LAST_PROFILE[label] = br.exec_time_ns
    return br.results


def kernel(x, edge_index, W0, att_src0, att_dst0, b0, W1, att_src1, att_dst1, b1):
    x = np.asarray(x, np.float32)
    edge_index = np.asarray(edge_index)
    d = _dims()
    N, NLOC, NP, NT = d["N"], d["NLOC"], d["NP"], d["NT"]

    prep = _prep_edges(edge_index, d)
    key = (prep["K2"], prep["K3"])
    if key not in _cache:
        _cache[key] = (build_l1(d), build_l2(d, prep["K2"]), build_l3(d, prep["K3"]))
    nc1, nc2, nc3 = _cache[key]

    A0 = _build_A0(np.asarray(att_src0), np.asarray(att_dst0))
    W0f = np.asarray(W0, np.float32)
    W0e = _bf16(np.concatenate([W0f, W0f @ A0], axis=1))
    W1f = np.asarray(W1, np.float32)
    was1 = W1f @ np.asarray(att_src1, np.float32).ravel()
    wad1 = W1f @ np.asarray(att_dst1, np.float32).ravel()
    W1e = _bf16(np.stack([*W1f.T, was1, wad1], axis=1))  # [256, 66]
    b0m1 = np.tile(np.asarray(b0, np.float32)[None, :] - 1.0, (128, 1))
    b1r = np.tile(np.asarray(b1, np.float32)[None, :], (128, 1))
    core_ids = list(range(NCORES))

    # launch 1
    xb = _bf16(x)
    in1 = []
    for c in range(NCORES):
        xT = np.zeros((d["F_IN"], NP), xb.dtype)
        xT[:, :NLOC] = xb[c * NLOC : (c + 1) * NLOC].T
        in1.append(dict(xT=xT, W0e=W0e))
    r1 = _run(nc1, in1, core_ids, "l1")

    tab0 = np.ascontiguousarray(
        np.concatenate(
            [r1[c]["t0T"][:, :NLOC] for c in range(NCORES)], axis=1
        ).T
    )  # [N, 256] bf16
    alf = np.concatenate([r1[c]["alT"][:, :NLOC] for c in range(NCORES)], axis=1)
    as_hi, as_lo = _hilo(alf[0:4].T)
    ad_hi, ad_lo = _hilo(alf[4:8].T)
    a0 = dict(as_hi=as_hi, as_lo=as_lo, ad_hi=ad_hi, ad_lo=ad_lo)

    in2 = [
        dict(
            _expand_l2(c, tab0, a0, prep),
            W1e=W1e,
            B0=_bf16(b0m1),
        )
        for c in range(NCORES)
    ]
    r2 = _run(nc2, in2, core_ids, "l2")

    # un-permute slot-major table1 columns -> node order
    tab1 = np.zeros((N, 64), r2[0]["t1T"].dtype)
    a1sh = np.zeros(N, np.float32)
    a1dh = np.zeros(N, np.float32)
    for c in range(NCORES):
        t1 = r2[c]["t1T"]  # [64, NP] slot-major
        a1c = r2[c]["a1T"]  # [2, NP]
        for s in range(NT):
            tl = prep["perm2"][c][s]
            n0 = tl * 128
            n1 = min(n0 + 128, NLOC)
            w = n1 - n0
            if w <= 0:
                continue
            tab1[c * NLOC + n0 : c * NLOC + n1] = t1[:, 128 * s : 128 * s + w].T
            a1sh[c * NLOC + n0 : c * NLOC + n1] = a1c[0, 128 * s : 128 * s + w]
            a1dh[c * NLOC + n0 : c * NLOC + n1] = a1c[1, 128 * s : 128 * s + w]
    s_hi, s_lo = _hilo(a1sh)
    d_hi, d_lo = _hilo(a1dh)
    a1 = dict(as_hi=s_hi, as_lo=s_lo, ad_hi=d_hi, ad_lo=d_lo)

    in3 = [
        dict(_expand_l3(c, tab1, a1, prep), B1=b1r)
        for c in range(NCORES)
    ]
    r3 = _run(nc3, in3, core_ids, "l3")

    out = np.zeros((N, 64), np.float32)
    for c in range(NCORES):
        o = r3[c]["out"]  # [NP, 64] slot-major
        for s in range(NT):
            tl = prep["perm3"][c][s]
            n0 = tl * 128
            n1 = min(n0 + 128, NLOC)
            w = n1 - n0
            if w <= 0:
                continue
            out[c * NLOC + n0 : c * NLOC + n1] = o[128 * s : 128 * s + w]
    return out


# revision 28
# speedup vs baseline: 1.4663x; 1.0181x over previous
"""Two-layer GAT (PyG-style GATConv x2) on 8 Trainium2 NeuronCores.

Design (v2, "host-expand"): nodes are sharded across the 8 cores by
destination. Between launches the HOST rearranges device-computed tables
(pure data movement: fancy-indexed row expansion per edge, sorting,
padding, hi/lo bf16 splits). All model arithmetic (matmuls, logit
add/leaky-relu/exp, softmax division, weighting, ELU, bias) runs on
device.

Rationale: per-edge SWDGE dma_gather costs ~8.3ns/edge of *serial* Q7
descriptor generation (~0.9ms/layer/core) - the measured bottleneck of
the v1 kernel. Pre-expanding edge payload rows on the host turns the
edge pass into dense sequential DMA + one-hot segment-sum matmuls.

Per-edge layout: edges are sorted by dst tile; each dst tile's edges are
padded to a multiple of 128 ("chunks"). Tiles are assigned to "slots" in
decreasing-count order per core so chunk counts align across the 8 SPMD
cores with minimal padding (the host un-permutes outputs).

Layer 1 packs 4 edges of the same dst node into one 260-wide row
(4 x (64 feats + w)), quartering the one-hot matmul count.

Three SPMD launches with host-side expansion between them:
  1. table0: h0^T = W0e^T @ x^T  -> feat-major table + per-node alphas
  2. layer-0 edges: stream payload/softmax/one-hot matmul -> ELU ->
     h1 = h0' @ W1e -> table1 (feat-major) + alphas
  3. layer-1 edges (quad-packed): same -> bias -> output shard
"""

import os

import numpy as np

import concourse.bacc as bacc
import concourse.mybir as mybir
from concourse import tile
from concourse.bass_utils import run_bass_kernel_spmd

fp32 = mybir.dt.float32
bf16 = mybir.dt.bfloat16
Alu = mybir.AluOpType
Act = mybir.ActivationFunctionType

NCORES = 8
NEG_SLOPE = 0.2
EPS = 1e-16
PAD_LOGIT = -30000.0
CPC = 16  # chunks per payload DMA call


def _dims():
    return dict(
        N=50000,
        NLOC=6250,
        NP=6272,  # padded to mult of 128
        NT=49,
        F_IN=256,
        HID=256,
        H=4,
        DH=64,
        C_OUT=64,
    )


# ---------------------------------------------------------------- launch 1


def build_l1(d):
    """h0^T = W0e^T @ x^T per core; W0e = [W0 | W0@A0] folds the per-node
    attention alphas into the same matmul. Outputs feat-major bf16 table
    plus fp32 alphas (host splits hi/lo)."""
    nc = bacc.Bacc(None, target_bir_lowering=False, debug=False)
    NP, F = d["NP"], d["F_IN"]

    xT = nc.dram_tensor("xT", [F, NP], bf16, kind="ExternalInput")
    W0e = nc.dram_tensor("W0e", [F, 264], bf16, kind="ExternalInput")
    t0T = nc.dram_tensor("t0T", [256, NP], bf16, kind="ExternalOutput")
    alT = nc.dram_tensor("alT", [8, NP], fp32, kind="ExternalOutput")

    TW = 512
    n_t = (NP + TW - 1) // TW

    with tile.TileContext(nc) as tc:
        with (
            tc.tile_pool(name="const", bufs=1) as cpool,
            tc.tile_pool(name="work", bufs=3) as pool,
            tc.tile_pool(name="psum", bufs=2, space="PSUM") as pp,
        ):
            w_sb = [
                cpool.tile([128, 264], bf16, tag=f"w{k}", name=f"w{k}")
                for k in range(2)
            ]
            for k in range(2):
                nc.sync.dma_start(w_sb[k][:], W0e[128 * k : 128 * (k + 1), :])

            for t in range(n_t):
                c0 = t * TW
                cw = min(TW, NP - c0)
                xt = [
                    pool.tile([128, TW], bf16, tag=f"xt{k}", name=f"xt{k}")
                    for k in range(2)
                ]
                for k in range(2):
                    nc.sync.dma_start(
                        xt[k][:, :cw], xT[128 * k : 128 * (k + 1), c0 : c0 + cw]
                    )
                for m in range(2):
                    ps = pp.tile([128, TW], fp32, tag=f"ps{m}", name=f"ps{m}")
                    for k in range(2):
                        nc.tensor.matmul(
                            ps[:, :cw],
                            w_sb[k][:, 128 * m : 128 * (m + 1)],
                            xt[k][:, :cw],
                            start=(k == 0),
                            stop=(k == 1),
                        )
                    ob = pool.tile([128, TW], bf16, tag=f"ob{m}", name=f"ob{m}")
                    nc.scalar.activation(ob[:, :cw], ps[:, :cw], Act.Copy)
                    nc.sync.dma_start(
                        t0T[128 * m : 128 * (m + 1), c0 : c0 + cw], ob[:, :cw]
                    )
                pa = pp.tile([8, TW], fp32, tag="pa", name="pa")
                for k in range(2):
                    nc.tensor.matmul(
                        pa[:, :cw],
                        w_sb[k][:, 256:264],
                        xt[k][:, :cw],
                        start=(k == 0),
                        stop=(k == 1),
                    )
                oa = pool.tile([8, TW], fp32, tag="oa", name="oa")
                nc.scalar.activation(oa[:, :cw], pa[:, :cw], Act.Copy)
                nc.sync.dma_start(alT[:, c0 : c0 + cw], oa[:, :cw])
    nc.compile()
    return nc


# ------------------------------------------------------------ edge machinery


def _logits_phase(nc, tc, d, L, NCH, ewb):
    """Batched per-edge softmax numerators: ewb = exp(lrelu(as+ad)) from
    hi/lo bf16 pieces, computed up-front for all chunks."""
    NBLK = 6
    nb = (NCH + NBLK - 1) // NBLK
    with tc.tile_pool(name="logit", bufs=2) as pool:
        for b in range(NBLK):
            b0 = b * nb
            bw = min(nb, NCH - b0)
            if bw <= 0:
                break
            lb = pool.tile([128, nb, 16], bf16, tag="lb", name="lb")
            nc.sync.dma_start(lb[:, :bw, :], L[:, b0 : b0 + bw, :])
            e8 = pool.tile([128, nb, 8], fp32, tag="e8", name="e8")
            nc.vector.tensor_tensor(
                e8[:, :bw, :], lb[:, :bw, 0:8], lb[:, :bw, 8:16], op=Alu.add
            )
            e4 = pool.tile([128, nb, 4], fp32, tag="e4", name="e4")
            nc.vector.tensor_tensor(
                e4[:, :bw, :], e8[:, :bw, 0:4], e8[:, :bw, 4:8], op=Alu.add
            )
            nc.vector.scalar_tensor_tensor(
                e4[:, :bw, :],
                e4[:, :bw, :],
                NEG_SLOPE,
                e4[:, :bw, :],
                op0=Alu.mult,
                op1=Alu.max,
            )
            nc.scalar.activation(ewb[:, b0 : b0 + bw, :], e4[:, :bw, :], Act.Exp)


def _edge_pass(nc, tc, d, P, OHD, Ks, ewb, fin, pp):
    """Stream pre-expanded 260-wide payload rows (4 blocks x (64 feats +
    w-slot)), weight by ewb, one-hot segment-sum into per-tile PSUM."""
    NCH = sum(Ks)

    with tc.tile_pool(name="edge", bufs=3) as pool:
        state = dict(ncalls=0, tiles={})

        def emit_call(call):
            c0 = call * CPC
            nch = min(CPC, NCH - c0)
            G = pool.tile([128, CPC, 264], bf16, tag="G", name="G", bufs=6)
            OH = pool.tile([128, CPC, 128], bf16, tag="OH", name="OH", bufs=6)
            # one-hots are host-built and streamed: a DVE is_equal build
            # measures ~2.2ns/elem (no fast uop + broadcast-port penalty),
            # so DMA is the cheaper engine for them
            nc.sync.dma_start(G[:, :nch, :], P[:, c0 : c0 + nch, :])
            nc.scalar.dma_start(OH[:, :nch, :], OHD[:, c0 : c0 + nch, :])
            g4 = G[:, :nch, :].rearrange("p c (h e) -> p c h e", e=66)
            wb = (
                ewb[:, c0 : c0 + nch, :]
                .unsqueeze(3)
                .broadcast_to([128, nch, 4, 66])
            )
            # payload w-slots are 1.0 from the host, so this multiply also
            # writes the per-block softmax-denominator columns
            nc.vector.tensor_tensor(g4, g4, wb, op=Alu.mult)
            return G, OH

        c = 0
        for s in range(len(Ks)):
            ps = pp.tile([128, 264], fp32, tag="ps", name="ps", bufs=4)
            for k in range(Ks[s]):
                call, cin = c // CPC, c % CPC
                if call >= state["ncalls"]:
                    state["tiles"][call] = emit_call(call)
                    state["ncalls"] = call + 1
                    state["tiles"].pop(call - 5, None)
                G, OH = state["tiles"][call]
                nc.tensor.matmul(
                    ps[:],
                    OH[:, cin, :],
                    G[:, cin, :],
                    start=(k == 0),
                    stop=(k == Ks[s] - 1),
                )
                c += 1
            fin(s, ps)


# ---------------------------------------------------------------- launch 2


def build_l2(d, Ks):
    """Layer-0 edge pass (softmax-div + bias + ELU fused in finalize),
    then table1^T = W1e^T @ h0'^T via a DMA-transpose round trip."""
    nc = bacc.Bacc(None, target_bir_lowering=False, debug=False)
    NP, NT, H = d["NP"], d["NT"], d["H"]
    NCH = sum(Ks)

    P = nc.dram_tensor("P", [128, NCH, 264], bf16, kind="ExternalInput")
    L = nc.dram_tensor("L", [128, NCH, 16], bf16, kind="ExternalInput")
    OHD = nc.dram_tensor("OHD", [128, NCH, 128], bf16, kind="ExternalInput")
    W1e = nc.dram_tensor("W1e", [256, 66], bf16, kind="ExternalInput")
    B0 = nc.dram_tensor("B0", [128, 256], bf16, kind="ExternalInput")
    t1T = nc.dram_tensor("t1T", [64, NP], bf16, kind="ExternalOutput")
    a1T = nc.dram_tensor("a1T", [2, NP], fp32, kind="ExternalOutput")

    with tile.TileContext(nc) as tc:
        with (
            tc.tile_pool(name="const", bufs=1) as cpool,
            tc.tile_pool(name="persist", bufs=1) as ipool,
            tc.tile_pool(name="fin", bufs=3) as fpool,
            tc.tile_pool(name="psum", bufs=1, space="PSUM") as pp,
        ):
            b0_sb = cpool.tile([128, 256], bf16)
            nc.sync.dma_start(b0_sb[:], B0[:])
            ewb = ipool.tile([128, NCH, 4], bf16)
            H0 = ipool.tile([128, NT, 256], bf16)

            _logits_phase(nc, tc, d, L, NCH, ewb)

            def fin0(s, ps):
                sb = fpool.tile([128, 264], fp32, tag="sb", name="sb")
                nc.scalar.activation(sb[:], ps[:], Act.Copy)
                pv = sb[:].rearrange("p (h e) -> p h e", h=H)
                dn = fpool.tile([128, H], fp32, tag="dn", name="dn")
                nc.vector.tensor_scalar_add(dn[:], pv[:, :, 64], EPS)
                rec = fpool.tile([128, H], fp32, tag="rec", name="rec")
                nc.vector.reciprocal(rec[:], dn[:])
                xp = fpool.tile([128, 256], bf16, tag="xp", name="xp")
                for h in range(H):
                    nc.scalar.activation(
                        xp[:, 64 * h : 64 * (h + 1)],
                        pv[:, h, 0:64],
                        Act.Copy,
                        scale=rec[:, h : h + 1],
                    )
                z = fpool.tile([128, 256], bf16, tag="z", name="z")
                nc.vector.tensor_tensor(z[:], xp[:], b0_sb[:], op=Alu.add)
                ex = fpool.tile([128, 256], fp32, tag="ex", name="ex")
                nc.scalar.activation(ex[:], z[:], Act.Exp, bias=1.0)
                m1 = fpool.tile([128, 256], fp32, tag="m1", name="m1")
                nc.vector.tensor_scalar_min(m1[:], ex[:], 1.0)
                nc.vector.scalar_tensor_tensor(
                    H0[:, s, :], z[:], -1.0, m1[:], op0=Alu.max, op1=Alu.add
                )

            _edge_pass(nc, tc, d, P, OHD, Ks, ewb, fin0, pp)

            with (
                tc.tile_pool(name="tb1", bufs=3) as tpool,
                tc.tile_pool(name="dram", bufs=1, space="DRAM") as dpool,
                tc.tile_pool(name="tb1psum", bufs=3, space="PSUM") as pp1,
            ):
                h0d = dpool.tile([NP, 256], bf16)
                nc.sync.dma_start(
                    h0d[:].rearrange("(t p) f -> p t f", p=128), H0[:, :, :]
                )
                h0T = [
                    ipool.tile([128, NP], bf16, tag=f"h0T{k}", name=f"h0T{k}")
                    for k in range(2)
                ]
                for k in range(2):
                    nc.sync.dma_start_transpose(
                        h0T[k][:], h0d[:, 128 * k : 128 * (k + 1)]
                    )
                w1_sb = [
                    cpool.tile([128, 66], bf16, tag=f"w1_{k}", name=f"w1_{k}")
                    for k in range(2)
                ]
                for k in range(2):
                    nc.sync.dma_start(w1_sb[k][:], W1e[128 * k : 128 * (k + 1), :])
                TW = 512
                for j in range((NP + TW - 1) // TW):
                    c0 = j * TW
                    cw = min(TW, NP - c0)
                    pt = pp1.tile([66, TW], fp32, tag="pt", name="pt")
                    for k in range(2):
                        nc.tensor.matmul(
                            pt[:, :cw],
                            w1_sb[k][:],
                            h0T[k][:, c0 : c0 + cw],
                            start=(k == 0),
                            stop=(k == 1),
                        )
                    tb = tpool.tile([64, TW], bf16, tag="tb", name="tb")
                    nc.scalar.activation(tb[:, :cw], pt[0:64, :cw], Act.Copy)
                    nc.sync.dma_start(t1T[:, c0 : c0 + cw], tb[:, :cw])
                    ab = tpool.tile([2, TW], fp32, tag="ab", name="ab")
                    nc.scalar.activation(ab[:, :cw], pt[64:66, :cw], Act.Copy)
                    nc.sync.dma_start(a1T[:, c0 : c0 + cw], ab[:, :cw])
    nc.compile()
    return nc


# ---------------------------------------------------------------- launch 3


def build_l3(d, Ks):
    """Layer-1 edge pass, quad-packed (4 same-dst edges per 260-wide row);
    finalize = sum quads, softmax-div, bias."""
    nc = bacc.Bacc(None, target_bir_lowering=False, debug=False)
    NP, C = d["NP"], d["C_OUT"]
    NCH = sum(Ks)

    P = nc.dram_tensor("P", [128, NCH, 264], bf16, kind="ExternalInput")
    L = nc.dram_tensor("L", [128, NCH, 16], bf16, kind="ExternalInput")
    OHD = nc.dram_tensor("OHD", [128, NCH, 128], bf16, kind="ExternalInput")
    B1 = nc.dram_tensor("B1", [128, C], fp32, kind="ExternalInput")
    out = nc.dram_tensor("out", [NP, C], fp32, kind="ExternalOutput")

    with tile.TileContext(nc) as tc:
        with (
            tc.tile_pool(name="const", bufs=1) as cpool,
            tc.tile_pool(name="persist", bufs=1) as ipool,
            tc.tile_pool(name="fin", bufs=3) as fpool,
            tc.tile_pool(name="psum", bufs=1, space="PSUM") as pp,
        ):
            b1_sb = cpool.tile([128, C], fp32)
            nc.sync.dma_start(b1_sb[:], B1[:])
            ewb = ipool.tile([128, NCH, 4], bf16)

            _logits_phase(nc, tc, d, L, NCH, ewb)

            def fin1(s, ps):
                sb = fpool.tile([128, 264], fp32, tag="sb", name="sb")
                nc.scalar.activation(sb[:], ps[:], Act.Copy)
                sv = sb[:].rearrange("p (q e) -> p q e", q=4)
                a01 = fpool.tile([128, 66], fp32, tag="a01", name="a01")
                nc.vector.tensor_tensor(a01[:], sv[:, 0, :], sv[:, 1, :], op=Alu.add)
                a23 = fpool.tile([128, 66], fp32, tag="a23", name="a23")
                nc.vector.tensor_tensor(a23[:], sv[:, 2, :], sv[:, 3, :], op=Alu.add)
                tot = fpool.tile([128, 66], fp32, tag="tot", name="tot")
                nc.vector.tensor_tensor(tot[:], a01[:], a23[:], op=Alu.add)
                dn = fpool.tile([128, 1], fp32, tag="dnq", name="dnq")
                nc.vector.tensor_scalar_add(dn[:], tot[:, 64:65], EPS)
                rec = fpool.tile([128, 1], fp32, tag="recq", name="recq")
                nc.vector.reciprocal(rec[:], dn[:])
                O = fpool.tile([128, C], fp32, tag="O", name="O")
                nc.vector.scalar_tensor_tensor(
                    O[:], tot[:, 0:64], rec[:], b1_sb[:], op0=Alu.mult, op1=Alu.add
                )
                nc.sync.dma_start(out[128 * s : 128 * (s + 1), :], O[:])

            _edge_pass(nc, tc, d, P, OHD, Ks, ewb, fin1, pp)
    nc.compile()
    return nc


# ------------------------------------------------------------ host plumbing


def _bf16(a):
    import ml_dtypes

    return np.asarray(a).astype(ml_dtypes.bfloat16)


def _hilo(a):
    """fp32 array -> (hi, lo) bf16 with hi+lo ~= a."""
    hi = _bf16(a)
    lo = _bf16(a - hi.astype(np.float32))
    return hi, lo


def _build_A0(att_src, att_dst):
    H, DH = att_src.shape
    A = np.zeros((H * DH, 2 * H), np.float32)
    for h in range(H):
        A[h * DH : (h + 1) * DH, h] = att_src[h]
        A[h * DH : (h + 1) * DH, H + h] = att_dst[h]
    return A


def _prep_edges(edge_index, d):
    """Per-core slot structure for both layers.

    l2 (per-edge): slots = dst tiles sorted by edge count (desc) per core;
    K2[s] = max over cores of ceil(count/128).
    l3 (quad): 4 same-dst edges per row; slots = tiles sorted by quad
    count. Returns per-core index arrays into the node tables.
    """
    N, NLOC, NT = d["N"], d["NLOC"], d["NT"]
    src = np.concatenate([edge_index[0], np.arange(N, dtype=np.int64)])
    dst = np.concatenate([edge_index[1], np.arange(N, dtype=np.int64)])
    core = dst // NLOC

    percore = []
    for c in range(NCORES):
        m = core == c
        s_c, t_c = src[m], dst[m] - c * NLOC
        order = np.argsort(t_c, kind="stable")
        percore.append((s_c[order], t_c[order]))

    # ---- layer-0 structure (per edge)
    counts2 = np.zeros((NCORES, NT), np.int64)
    for c in range(NCORES):
        counts2[c] = np.bincount(percore[c][1] // 128, minlength=NT)
    perm2 = np.argsort(-counts2, axis=1, kind="stable")  # [core, slot] -> tile
    sorted2 = -np.sort(-counts2, axis=1)
    K2 = tuple(int(k) for k in np.ceil(sorted2.max(axis=0) / 128).astype(int))
    NCH2 = sum(K2)
    base2 = np.concatenate([[0], np.cumsum(np.array(K2) * 128)])

    l2 = []
    for c in range(NCORES):
        s_c, t_c = percore[c]
        tile_of = t_c // 128
        EP = NCH2 * 128
        gsrc = np.zeros(EP, np.int64)
        gdst = np.zeros(EP, np.int64)
        rr = np.full(EP, -1.0, np.float32)
        pad = np.ones(EP, bool)
        offs = np.concatenate([[0], np.cumsum(counts2[c][perm2[c]])])
        # edges are tile-sorted; index ranges per tile:
        tstart = np.concatenate([[0], np.cumsum(counts2[c])])
        for s in range(NT):
            tl = perm2[c][s]
            n = counts2[c][tl]
            sl = slice(tstart[tl], tstart[tl] + n)
            b = base2[s]
            gsrc[b : b + n] = s_c[sl]
            gdst[b : b + n] = t_c[sl] + c * NLOC
            rr[b : b + n] = (t_c[sl] - 128 * tl).astype(np.float32)
            pad[b : b + n] = False
        l2.append(dict(gsrc=gsrc, gdst=gdst, rr=rr, pad=pad))

    # ---- layer-1 structure (quads)
    counts3 = np.zeros((NCORES, NT), np.int64)
    quads_pc = []
    for c in range(NCORES):
        s_c, t_c = percore[c]
        deg = np.bincount(t_c, minlength=NLOC)
        nq = (deg + 3) // 4  # quads per node
        counts3[c] = np.add.reduceat(
            nq, np.arange(0, NLOC, 128)
        )
        quads_pc.append((s_c, t_c, deg, nq))
    perm3 = np.argsort(-counts3, axis=1, kind="stable")
    sorted3 = -np.sort(-counts3, axis=1)
    K3 = tuple(int(k) for k in np.ceil(sorted3.max(axis=0) / 128).astype(int))
    NCH3 = sum(K3)
    base3 = np.concatenate([[0], np.cumsum(np.array(K3) * 128)])

    l3 = []
    for c in range(NCORES):
        s_c, t_c, deg, nq = quads_pc[c]
        EP = NCH3 * 128
        qsrc = np.zeros((EP, 4), np.int64)
        qdst = np.zeros(EP, np.int64)
        rr = np.full(EP, -1.0, np.float32)
        pad = np.ones((EP, 4), bool)
        estart = np.concatenate([[0], np.cumsum(deg)])
        qstart_tile = np.concatenate(
            [[0], np.cumsum(counts3[c])]
        )  # quad offset per tile (in tile order)
        for s in range(NT):
            tl = perm3[c][s]
            b = base3[s]
            q = 0
            n0 = tl * 128
            n1 = min(n0 + 128, NLOC)
            for node in range(n0, n1):
                dg = deg[node]
                if dg == 0:
                    continue
                e0 = estart[node]
                nqn = nq[node]
                rows = b + q + np.arange(nqn)
                rr[rows] = float(node - n0)
                qdst[rows] = node + c * NLOC
                es = s_c[e0 : e0 + dg]
                full = np.zeros(nqn * 4, np.int64)
                full[:dg] = es
                qsrc[rows] = full.reshape(nqn, 4)
                pd = np.ones(nqn * 4, bool)
                pd[:dg] = False
                pad[rows] = pd.reshape(nqn, 4)
                q += nqn
        l3.append(dict(qsrc=qsrc, qdst=qdst, rr=rr, pad=pad))

    return dict(K2=K2, K3=K3, perm2=perm2, perm3=perm3, l2=l2, l3=l3)


_EYEP = None


def _oh_rows(rr):
    """rr [EP] (float, -1 = padding) -> one-hot rows [EP, 128] bf16."""
    global _EYEP
    if _EYEP is None:
        _EYEP = np.zeros((129, 128), np.float32)
        _EYEP[:128] = np.eye(128, dtype=np.float32)
        _EYEP = _bf16(_EYEP)
    idx = rr.astype(np.int64)
    idx[idx < 0] = 128
    return _EYEP[idx]


def _pack_pm(a, nch):
    """[EP, W] row-major -> [128, nch, W] partition-major contiguous."""
    W = a.shape[1]
    return np.ascontiguousarray(a.reshape(nch, 128, W).transpose(1, 0, 2))


def _expand_l2(core_idx, tab0, a0, prep):
    """Per-core launch-2 inputs from full node tables (pure gather)."""
    K2 = prep["K2"]
    NCH = sum(K2)
    e = prep["l2"][core_idx]
    gsrc, gdst, pad = e["gsrc"], e["gdst"], e["pad"]
    EP = NCH * 128
    rows = tab0[gsrc]  # [EP, 256] bf16
    P = np.zeros((EP, 264), rows.dtype)
    pv = P.reshape(EP, 4, 66)
    pv[:, :, 0:64] = rows.reshape(EP, 4, 64)
    pv[:, :, 64] = 1.0  # weighting writes w into these denominator slots
    as_hi, as_lo = a0["as_hi"][gsrc], a0["as_lo"][gsrc]
    ad_hi, ad_lo = a0["ad_hi"][gdst], a0["ad_lo"][gdst]
    L = np.concatenate([as_hi, as_lo, ad_hi, ad_lo], axis=1)
    L[pad, 0:4] = PAD_LOGIT
    return dict(
        P=_pack_pm(P, NCH),
        L=_pack_pm(L, NCH),
        OHD=_pack_pm(_oh_rows(e["rr"]), NCH),
    )


def _expand_l3(core_idx, tab1, a1, prep):
    K3 = prep["K3"]
    NCH = sum(K3)
    e = prep["l3"][core_idx]
    qsrc, qdst, pad = e["qsrc"], e["qdst"], e["pad"]
    EP = NCH * 128
    P = np.zeros((EP, 264), tab1.dtype)
    pv = P.reshape(EP, 4, 66)
    for j in range(4):
        pv[:, j, 0:64] = tab1[qsrc[:, j]]
    pv[:, :, 64] = 1.0  # weighting writes w into these denominator slots
    as_hi = a1["as_hi"][qsrc]  # [EP, 4]
    as_lo = a1["as_lo"][qsrc]
    ad_hi = np.repeat(a1["ad_hi"][qdst][:, None], 4, axis=1)
    ad_lo = np.repeat(a1["ad_lo"][qdst][:, None], 4, axis=1)
    L = np.concatenate([as_hi, as_lo, ad_hi, ad_lo], axis=1)
    L[:, 0:4][pad] = PAD_LOGIT
    return dict(
        P=_pack_pm(P, NCH),
        L=_pack_pm(L, NCH),
        OHD=_pack_pm(_oh_rows(e["rr"]), NCH),
    )


_cache = {}
LAST_PROFILE = {}


def _run(nc, in_maps, core_ids, label):
    trace = bool(int(os.environ.get("GAT_PROFILE", "0")))
    if trace:
        try:
            import sys

            import profile_hook

            profile_hook.install()
            import concourse.bass_utils as bu

            bu.upload_artifacts = lambda tmpdir: "local://skipped"
            tdir = f"/tmp/gat_trace_{label}"
            os.makedirs(tdir, exist_ok=True)
            for f in os.listdir(tdir):
                os.unlink(os.path.join(tdir, f))
            br = run_bass_kernel_spmd(nc, in_maps, core_ids, trace=True, tmpdir=tdir)
            LAST_PROFILE[label] = br.exec_time_ns
            return br.results
        except Exception as e:  # fall back to untraced
            print(f"traced run failed ({e!r}); untraced retry", file=sys.stderr)
    br = run_bass_kernel_spmd(nc, in_maps, core_ids)
    LAST_PROFILE[label] = br.exec_time_ns
    return br.results


def kernel(x, edge_index, W0, att_src0, att_dst0, b0, W1, att_src1, att_dst1, b1):
    x = np.asarray(x, np.float32)
    edge_index = np.asarray(edge_index)
    d = _dims()
    N, NLOC, NP, NT = d["N"], d["NLOC"], d["NP"], d["NT"]

    prep = _prep_edges(edge_index, d)
    key = (prep["K2"], prep["K3"])
    if key not in _cache:
        _cache[key] = (build_l1(d), build_l2(d, prep["K2"]), build_l3(d, prep["K3"]))
    nc1, nc2, nc3 = _cache[key]

    A0 = _build_A0(np.asarray(att_src0), np.asarray(att_dst0))
    W0f = np.asarray(W0, np.float32)
    W0e = _bf16(np.concatenate([W0f, W0f @ A0], axis=1))
    W1f = np.asarray(W1, np.float32)
    was1 = W1f @ np.asarray(att_src1, np.float32).ravel()
    wad1 = W1f @ np.asarray(att_dst1, np.float32).ravel()
    W1e = _bf16(np.stack([*W1f.T, was1, wad1], axis=1))  # [256, 66]
    b0m1 = np.tile(np.asarray(b0, np.float32)[None, :] - 1.0, (128, 1))
    b1r = np.tile(np.asarray(b1, np.float32)[None, :], (128, 1))
    core_ids = list(range(NCORES))

    # launch 1
    xb = _bf16(x)
    in1 = []
    for c in range(NCORES):
        xT = np.zeros((d["F_IN"], NP), xb.dtype)
        xT[:, :NLOC] = xb[c * NLOC : (c + 1) * NLOC].T
        in1.append(dict(xT=xT, W0e=W0e))
    r1 = _run(nc1, in1, core_ids, "l1")

    tab0 = np.ascontiguousarray(
        np.concatenate(
            [r1[c]["t0T"][:, :NLOC] for c in range(NCORES)], axis=1
        ).T
    )  # [N, 256] bf16
    alf = np.concatenate([r1[c]["alT"][:, :NLOC] for c in range(NCORES)], axis=1)
    as_hi, as_lo = _hilo(alf[0:4].T)
    ad_hi, ad_lo = _hilo(alf[4:8].T)
    a0 = dict(as_hi=as_hi, as_lo=as_lo, ad_hi=ad_hi, ad_lo=ad_lo)

    in2 = [
        dict(
            _expand_l2(c, tab0, a0, prep),
            W1e=W1e,
            B0=_bf16(b0m1),
        )
        for c in range(NCORES)
    ]
    r2 = _run(nc2, in2, core_ids, "l2")

    # un-permute slot-major table1 columns -> node order
    tab1 = np.zeros((N, 64), r2[0]["t1T"].dtype)
    a1sh = np.zeros(N, np.float32)
    a1dh = np.zeros(N, np.float32)
    for c in range(NCORES):
        t1 = r2[c]["t1T"]  # [64, NP] slot-major
        a1c = r2[c]["a1T"]  # [2, NP]
        for s in range(NT):
            tl = prep["perm2"][c][s]
            n0 = tl * 128
            n1 = min(n0 + 128, NLOC)
            w = n1 - n0
            if w <= 0:
                continue
            tab1[c * NLOC + n0 : c * NLOC + n1] = t1[:, 128 * s : 128 * s + w].T
            a1sh[c * NLOC + n0 : c * NLOC + n1] = a1c[0, 128 * s : 128 * s + w]
            a1dh[c * NLOC + n0 : c * NLOC + n1] = a1c[1, 128 * s : 128 * s + w]
    s_hi, s_lo = _hilo(a1sh)
    d_hi, d_lo = _hilo(a1dh)
    a1 = dict(as_hi=s_hi, as_lo=s_lo, ad_hi=d_hi, ad_lo=d_lo)

    in3 = [
        dict(_expand_l3(c, tab1, a1, prep), B1=b1r)
        for c in range(NCORES)
    ]
    r3 = _run(nc3, in3, core_ids, "l3")

    out = np.zeros((N, 64), np.float32)
    for c in range(NCORES):
        o = r3[c]["out"]  # [NP, 64] slot-major
        for s in range(NT):
            tl = prep["perm3"][c][s]
            n0 = tl * 128
            n1 = min(n0 + 128, NLOC)
            w = n1 - n0
            if w <= 0:
                continue
            out[c * NLOC + n0 : c * NLOC + n1] = o[128 * s : 128 * s + w]
    return out
